# revision 40
# baseline (speedup 1.0000x reference)
"""Conformer layer on 8 Trainium2 NeuronCores.

Sharding: core c handles batch b=c//2, token half sc=c%2 (512 tokens).
All stages are token-parallel; attention computes ALL 8 heads for the
core's own 544-token window (512 own + 16-token halos), so no collective
is needed after attention: the conv-module halo comes straight from the
locally computed window. Cross-core traffic: one pair AllGather of the
post-FFN1 hidden state and four tiny 8-way AllReduces of per-chunk
BatchNorm statistics.

Activations are channels-first ([channel, token]); matmuls run in
float32r (full PE rate at moving >= 256). The rel-position term bd is a
banded matmul bounced through DRAM in bf16 with a row-stride mismatch
that realizes rel-shift+transpose as one strided read; it is accumulated
onto the score PSUM by a bf16 identity matmul. The depthwise conv runs
as 31 accumulating diagonal bf16 matmuls per channel chunk. FFN weights
stream j-sliced so matmuls start before the full weight load completes.
"""

import numpy as np

import concourse.bass as bass
import concourse.mybir as mybir
import concourse.tile as tile
from concourse import bacc
from concourse.bass import ds, ts
from concourse.bass_utils import run_bass_kernel_spmd
from contextlib import ExitStack

F32 = mybir.dt.float32
F32R = mybir.dt.float32r
BF16 = mybir.dt.bfloat16
AF = mybir.ActivationFunctionType
ALU = mybir.AluOpType

D, DFF, H, DK, KCONV = 512, 2048, 8, 64, 31
B, T = 4, 1024
EPS = 1e-5
HT = 512            # own tokens per core
WQ = 544            # attention/conv window: 16 + 512 + 16
FR = 1056           # padded frame: 16 + 1024 + 16
PW = 1568           # local (pre-shifted) positional width
BRS = 1280          # bd dram row stride (elements)
BDSZ = 5 * 128 * BRS
TAILOFF = 4 * 128 * (BRS - 1) + 96
NCORES = 8

PAIRS = [[0, 1], [2, 3], [4, 5], [6, 7]]
ALLG = [[0, 1, 2, 3, 4, 5, 6, 7]]

# bd q-chunks: (q0, qw, band_base, band_pi_width)
BDCH = [(0, 128, 416, 384), (128, 128, 288, 384), (256, 128, 160, 384),
        (384, 128, 32, 384), (512, 32, 0, 352)]


def _r(ap):
    return ap.bitcast(F32R)


def _emit(nc):
    def inp(name, shape, dt=F32):
        return nc.dram_tensor(name, list(shape), dt, kind="ExternalInput")

    x_d = inp("x_loc", (4, 128, HT), F32R)
    wf1_d = inp("wf1", (16, 128, D), F32R); bf1_d = inp("bf1", (128, 16))
    wf2_d = inp("wf2", (16, 128, D), F32R); bf2_d = inp("bf2", (128, 4))
    wq_d = inp("wq", (4, 128, D), F32R); bq_d = inp("bq", (128, 4))
    dqv_d = inp("dqv", (128, 4))
    wk_d = inp("wk", (4, 128, D), F32R); bk_d = inp("bk", (128, 4))
    wv_d = inp("wv", (4, 128, D), F32R); bv_d = inp("bvrow", (1, D))
    wp_d = inp("wp", (4, 128, D), F32R)
    wo_d = inp("wo", (4, 128, D), F32R); bo_d = inp("bo", (128, 4))
    posT_d = inp("posT", (4, 128, PW), F32R)
    pw1_d = inp("pw1", (4, 128, 1024), F32R); bpw1_d = inp("bpw1", (128, 8))
    dwdg_d = inp("dwdiag", (4, KCONV, 128, 128), BF16)
    bng_d = inp("bng", (128, 4)); bnb_d = inp("bnb", (128, 4))
    pw2_d = inp("pw2", (4, 128, D), F32R); bpw2_d = inp("bpw2", (128, 4))
    cmask_d = inp("cmask", (1, WQ))
    wg1_d = inp("wg1", (16, 128, D), F32R); bg1_d = inp("bg1", (128, 16))
    wg2_d = inp("wg2", (16, 128, D), F32R); bg2_d = inp("bg2", (128, 4))
    g5_d = inp("g5", (128, 4)); b5_d = inp("b5", (128, 4))
    onc_d = inp("onc", (128, 1), F32R)
    idb_d = inp("idb", (128, 128), BF16)
    onv_d = inp("onv", (128, 64), BF16)

    out_d = nc.dram_tensor("out_loc", [4, 128, HT], F32, kind="ExternalOutput")

    cc_h_in = nc.dram_tensor("cc_h_in", [4, 128, HT], F32R)
    cc_h_out = nc.dram_tensor("cc_h_out", [8, 128, HT], F32R)
    cc_bn_in = nc.dram_tensor("cc_bn_in", [128, 8], F32)
    cc_bn_out = nc.dram_tensor("cc_bn_out", [128, 8], F32)
    bd_d = [nc.dram_tensor(f"bd_{h}", [BDSZ], BF16) for h in range(H)]

    ln_uid = [0]

    with tile.TileContext(nc) as tc, ExitStack() as ctx:
        const = ctx.enter_context(tc.tile_pool(name="const", bufs=1))
        ones_c = const.tile([128, 1], F32R)
        nc.sync.dma_start(out=ones_c[:], in_=onc_d[:])
        eps1 = const.tile([1, 1], F32); nc.vector.memset(eps1[:], EPS)
        epsP = const.tile([128, 1], F32); nc.vector.memset(epsP[:], EPS)
        identb = const.tile([128, 128], BF16)
        nc.sync.dma_start(out=identb[:], in_=idb_d[:])

        late = ctx.enter_context(tc.tile_pool(name="late", bufs=1))
        h2w = late.tile([128, 4, WQ], F32R)   # post-attention hidden window
        h3_sb = late.tile([128, 4, HT], F32R)    # post-conv hidden
        h4_sb = late.tile([128, 4, HT], F32R)    # post-FFN2 hidden

        actpool = tc.tile_pool(name="act", bufs=1)
        act = actpool.__enter__()
        # full padded hidden frame (post-FFN1), 16 zero tokens on both ends
        hfe_sb = act.tile([128, 4, FR], F32R)
        nc.vector.memset(hfe_sb[:, :, 0:16].bitcast(F32), 0.0)
        nc.vector.memset(hfe_sb[:, :, 16 + T:].bitcast(F32), 0.0)

        pid = nc.vector.partition_id()
        woff = (pid % 2) * 512      # window start in frame coords

        # ---------- LN helper (channels-first) ----------
        def emit_ln(x4, out4, W, blocks, sbp, post=None):
            ln_uid[0] += 1
            with tc.tile_pool(name=f"lnps{ln_uid[0]}", bufs=1,
                              space="PSUM") as lnps:
                for b0, bw in blocks:
                    x2 = sbp.tile([128, 4, bw], F32R, tag="ln_sq")
                    for c in range(4):
                        nc.vector.tensor_mul(x2[:, c, :],
                                             x4[:, c, b0:b0 + bw],
                                             x4[:, c, b0:b0 + bw])
                    pss = lnps.tile([1, bw], F32, tag="lns")
                    psq = lnps.tile([1, bw], F32, tag="lnq")
                    for c in range(4):
                        nc.tensor.matmul(pss[:], _r(ones_c[:]),
                                         _r(x4[:, c, b0:b0 + bw]),
                                         start=(c == 0), stop=(c == 3))
                    for c in range(4):
                        nc.tensor.matmul(psq[:], _r(ones_c[:]),
                                         _r(x2[:, c, :]),
                                         start=(c == 0), stop=(c == 3))
                    mng = sbp.tile([1, bw], F32, tag="ln_m")
                    nc.scalar.activation(mng[:], pss[:], AF.Copy,
                                         scale=-1.0 / D)
                    e2 = sbp.tile([1, bw], F32, tag="ln_e2")
                    nc.scalar.activation(e2[:], psq[:], AF.Copy, scale=1.0 / D)
                    var = sbp.tile([1, bw], F32, tag="ln_var")
                    nc.vector.tensor_mul(var[:], mng[:], mng[:])
                    nc.vector.tensor_sub(var[:], e2[:], var[:])
                    sd = sbp.tile([1, bw], F32, tag="ln_sd")
                    nc.scalar.activation(sd[:], var[:], AF.Sqrt, bias=eps1[:])
                    rec = sbp.tile([1, bw], F32, tag="ln_rs")
                    scr = sbp.tile([1, bw], F32, tag="ln_scr")
                    nc.vector.reciprocal_approx_accurate(rec[:], sd[:], scr[:])
                    nmr = sbp.tile([1, bw], F32, tag="ln_nm")
                    nc.vector.tensor_mul(nmr[:], mng[:], rec[:])
                    rb_t = sbp.tile([128, bw], F32, tag="ln_rb")
                    nc.gpsimd.partition_broadcast(rb_t[:], rec[:])
                    nb_t = sbp.tile([128, bw], F32, tag="ln_nb")
                    nc.gpsimd.partition_broadcast(nb_t[:], nmr[:])
                    for c in range(4):
                        o = out4[:, c, b0:b0 + bw]
                        nc.vector.tensor_mul(o, x4[:, c, b0:b0 + bw], rb_t[:])
                        nc.vector.tensor_add(o, o, nb_t[:])
                        if post is not None:
                            post(c, b0, bw)

        # ---------- FFN helper (weights already in SBUF, j-sliced) ----------
        def emit_ffn(xn, xres, out, w1, b1, w2, b2, pref):
            with tc.tile_pool(name=pref + "t", bufs=3) as tp, \
                 tc.tile_pool(name=pref + "ps", bufs=2, space="PSUM") as psp, \
                 tc.tile_pool(name=pref + "ph", bufs=1, space="PSUM") as php:
                psh = php.tile([128, 4, HT], F32)
                for j in range(16):
                    psy = psp.tile([128, HT], F32, tag="psy")
                    for c in range(4):
                        nc.tensor.matmul(psy[:], _r(w1[:, j, ts(c, 128)]),
                                         _r(xn[:, c, :]),
                                         start=(c == 0), stop=(c == 3))
                    y1 = tp.tile([128, HT], F32R, tag="y1")
                    nc.scalar.activation(y1[:], psy[:], AF.Silu,
                                         bias=b1[:, j:j + 1])
                    for f in range(4):
                        nc.tensor.matmul(psh[:, f, :],
                                         _r(w2[:, j, ts(f, 128)]), _r(y1[:]),
                                         start=(j == 0), stop=(j == 15))
                for c in range(4):
                    nc.vector.scalar_tensor_tensor(
                        out=out[:, c, :], in0=psh[:, c, :],
                        scalar=b2[:, c:c + 1], in1=xres[:, c, :],
                        op0=ALU.add, op1=ALU.add)

        # ================= Stage A: FFN1 on local tokens =================
        with tc.tile_pool(name="f1w", bufs=1) as f1w, \
             tc.tile_pool(name="stA", bufs=1) as stA, \
             tc.tile_pool(name="stAt", bufs=2) as stAt:
            x_sb = stA.tile([128, 4, HT], F32R)
            for c in range(4):
                nc.sync.dma_start(out=x_sb[:, c, :], in_=x_d[c])
            w1 = f1w.tile([128, 16, D], F32R)
            w2 = f1w.tile([128, 16, D], F32R)
            for j in range(16):
                nc.sync.dma_start(out=w1[:, j, :], in_=wf1_d[j])
                nc.scalar.dma_start(out=w2[:, j, :], in_=wf2_d[j])
            b1 = f1w.tile([128, 16], F32)
            nc.sync.dma_start(out=b1[:], in_=bf1_d[:])
            b2 = f1w.tile([128, 4], F32)
            nc.sync.dma_start(out=b2[:], in_=bf2_d[:])
            h_sb = stA.tile([128, 4, HT], F32R)
            xn1 = stA.tile([128, 4, HT], F32R)
            emit_ln(x_sb[:], xn1[:], HT, [(0, HT)], stAt)
            emit_ffn(xn1[:], x_sb[:], h_sb[:], w1, b1[:], w2, b2[:], "f1")
            for c in range(4):
                nc.sync.dma_start(out=cc_h_in[c], in_=h_sb[:, c, :])

        nc.gpsimd.collective_compute(
            "AllGather", ALU.bypass, ins=[cc_h_in[:]], outs=[cc_h_out[:]],
            replica_groups=PAIRS)

        # ============ attention (all 8 heads, own 544-token window) =======
        attpool = tc.tile_pool(name="attp", bufs=1)
        attp = attpool.__enter__()
        p_sb = attp.tile([128, 4, PW], F32R)
        # ---- p-projection (independent of AllGather; overlaps it) ----
        with tc.tile_pool(name="ppp", bufs=2, space="PSUM") as ppp, \
             tc.tile_pool(name="atw", bufs=1) as atw:
            wp_sb = atw.tile([128, 4, D], F32R, tag="wp")
            nc.sync.dma_start(out=wp_sb[:],
                              in_=wp_d[:].rearrange("c p w -> p c w"))
            pos_sb = atw.tile([128, 4, PW], F32R, tag="pos")
            for c in range(4):
                nc.scalar.dma_start(out=pos_sb[:, c, :], in_=posT_d[c])
            for hp in range(4):
                psp_t = ppp.tile([128, 3, 512], F32, tag="psp")
                psp_u = ppp.tile([128, 32], F32, tag="pspu")
                for c in range(4):
                    st, sp = (c == 0), (c == 3)
                    for pc in range(3):
                        nc.tensor.matmul(
                            psp_t[:, pc, :], _r(wp_sb[:, c, ts(hp, 128)]),
                            _r(pos_sb[:, c, ts(pc, 512)]), start=st, stop=sp)
                    nc.tensor.matmul(
                        psp_u[:], _r(wp_sb[:, c, ts(hp, 128)]),
                        _r(pos_sb[:, c, 1536:PW]), start=st, stop=sp)
                for pc in range(3):
                    if hp % 2 == 0:
                        nc.vector.tensor_copy(
                            p_sb[:, hp, ts(pc, 512)],
                            psp_t[:, pc, :])
                    else:
                        nc.scalar.activation(
                            p_sb[:, hp, ts(pc, 512)],
                            psp_t[:, pc, :], AF.Copy)
                nc.vector.tensor_copy(p_sb[:, hp, 1536:PW],
                                      psp_u[:])

        # ---- gather h_full into frame, LN2 over padded frame ----
        for half in range(2):
            for c in range(4):
                nc.sync.dma_start(
                    out=hfe_sb[:, c, 16 + half * HT:16 + (half + 1) * HT],
                    in_=cc_h_out[half * 4 + c])
        xn2p = attp.tile([128, 4, FR], F32R)
        with tc.tile_pool(name="stC", bufs=1) as stC, \
             tc.tile_pool(name="stCt", bufs=2) as stCt:
            emit_ln(hfe_sb[:], xn2p[:], FR,
                    [(0, 352), (352, 352), (704, 352)], stCt)

            # window copy for q (dynamic slice -> static tile)
            xn2w = stC.tile([128, 4, WQ], F32R)
            for c in range(4):
                nc.vector.tensor_copy(xn2w[:, c, :],
                                      xn2p[:, c, ds(woff, WQ)])

            # ---- q, qv, k, v projections ----
            q_sb = attp.tile([128, 4, WQ], F32R)
            qv_sb = attp.tile([128, 4, WQ], F32R)
            k_sb = attp.tile([128, 4, T], F32R)
            v65 = attp.tile([128, 8, 8, 65], BF16)
            nc.sync.dma_start(out=v65[:, :, :, 64:65],
                              in_=onv_d[:].rearrange("p (a b) -> p a b", a=8))
            with tc.tile_pool(name="qkp", bufs=2, space="PSUM") as psp:
                wq_sb = stC.tile([128, 4, D], F32R)
                wk_sb = stC.tile([128, 4, D], F32R)
                wv_sb = stC.tile([128, 4, D], F32R)
                nc.sync.dma_start(out=wq_sb[:],
                                  in_=wq_d[:].rearrange("c p w -> p c w"))
                nc.scalar.dma_start(out=wk_sb[:],
                                    in_=wk_d[:].rearrange("c p w -> p c w"))
                nc.sync.dma_start(out=wv_sb[:],
                                  in_=wv_d[:].rearrange("c p w -> p c w"))
                bq_sb = stC.tile([128, 4], F32)
                nc.sync.dma_start(out=bq_sb[:], in_=bq_d[:])
                bk_sb = stC.tile([128, 4], F32)
                nc.sync.dma_start(out=bk_sb[:], in_=bk_d[:])
                dqv_sb = stC.tile([128, 4], F32)
                nc.sync.dma_start(out=dqv_sb[:], in_=dqv_d[:])
                bv_sb = stC.tile([1, D], F32)
                nc.sync.dma_start(out=bv_sb[:], in_=bv_d[:])
                bvb_sb = stC.tile([128, D], F32)
                nc.gpsimd.partition_broadcast(bvb_sb[:], bv_sb[:])
                for hp in range(4):
                    psq = psp.tile([128, 2, 512], F32, tag="psq")
                    for c in range(4):
                        st, sp = (c == 0), (c == 3)
                        for h2 in range(2):
                            nc.tensor.matmul(psq[:, h2, 0:272],
                                             _r(wq_sb[:, c, ts(hp, 128)]),
                                             _r(xn2w[:, c, h2 * 272:
                                                     (h2 + 1) * 272]),
                                             start=st, stop=sp,
                                             skip_group_check=True)
                    nc.vector.tensor_scalar_add(
                        q_sb[:, hp, :].rearrange("p (a w) -> p a w", a=2),
                        psq[:, :, 0:272], bq_sb[:, hp:hp + 1])
                    nc.vector.tensor_scalar_add(qv_sb[:, hp, :],
                                                q_sb[:, hp, :],
                                                dqv_sb[:, hp:hp + 1])
                    for th in range(2):
                        psk = psp.tile([128, 512], F32, tag="psk")
                        for c in range(4):
                            nc.tensor.matmul(
                                psk[:], _r(wk_sb[:, c, ts(hp, 128)]),
                                _r(xn2p[:, c, 16 + th * 512:16 + th * 512 + 512]),
                                start=(c == 0), stop=(c == 3))
                        nc.vector.tensor_scalar_add(
                            k_sb[:, hp, th * 512:(th + 1) * 512], psk[:],
                            bk_sb[:, hp:hp + 1])
                for tq in range(8):
                    psv = psp.tile([128, 512], F32, tag="psv")
                    for c in range(4):
                        nc.tensor.matmul(
                            psv[:], _r(xn2p[:, c, 16 + tq * 128:16 + (tq + 1) * 128]),
                            _r(wv_sb[:, c, :]),
                            start=(c == 0), stop=(c == 3))
                    nc.vector.tensor_add(
                        v65[:, tq, :, 0:64],
                        psv[:].rearrange("p (h d) -> p h d", h=8),
                        bvb_sb[:].rearrange("p (h d) -> p h d", h=8))

        # ---- per-head: banded bd -> DRAM bounce -> scores -> av ----
        o_h = attp.tile([128, 4, WQ], F32R)
        with tc.tile_pool(name="bds", bufs=2) as bdp, \
             tc.tile_pool(name="bdsh", bufs=2) as shp, \
             tc.tile_pool(name="atp", bufs=2) as atp, \
             tc.tile_pool(name="atn", bufs=1) as atn, \
             tc.tile_pool(name="psb", bufs=2, space="PSUM") as psb, \
             tc.tile_pool(name="pss", bufs=2, space="PSUM") as pss, \
             tc.tile_pool(name="psav", bufs=1, space="PSUM") as psav:

            def emit_bd(h):
                hc, hr = h // 2, 64 * (h % 2)
                for ci, (q0, qw, m0, pw) in enumerate(BDCH):
                    bdw = bdp.tile([128, 3, 384], BF16, tag="bdw")
                    for pi in range(3):
                        psB = psb.tile([128, 384], F32, tag="psB")
                        nc.tensor.matmul(
                            psB[0:qw, 0:pw],
                            _r(qv_sb[hr:hr + 64, hc, q0:q0 + qw]),
                            _r(p_sb[hr:hr + 64, hc,
                                    m0 + pi * pw:m0 + (pi + 1) * pw]),
                            start=True, stop=True)
                        if pi != 1:
                            nc.vector.tensor_copy(bdw[0:qw, pi, 0:pw],
                                                  psB[0:qw, 0:pw])
                        else:
                            nc.scalar.activation(bdw[0:qw, pi, 0:pw],
                                                 psB[0:qw, 0:pw], AF.Copy)
                    off = ci * 128 * (BRS - 1) if qw == 128 else TAILOFF
                    dst = bass.AP(tensor=bd_d[h], offset=off,
                                  ap=[[BRS, qw], [1, 3 * pw]])
                    nc.gpsimd.dma_start(
                        out=dst, in_=bdw[0:qw, :, 0:pw])

            def emit_scores(h, bdsh):
                hc, hr = h // 2, 64 * (h % 2)
                psA = psav.tile([65, 2, 512], F32, tag="psAV")
                for kc in range(8):
                    psS = pss.tile([128, 2, 512], F32, tag="psS")
                    for h2 in range(2):
                        sl = slice(h2 * 272, (h2 + 1) * 272)
                        nc.tensor.matmul(
                            psS[:, h2, 0:272],
                            _r(k_sb[hr:hr + 64, hc, ts(kc, 128)]),
                            _r(q_sb[hr:hr + 64, hc, sl]),
                            start=True, stop=False, skip_group_check=True)
                        nc.tensor.matmul(
                            psS[:, h2, 0:272], identb[:], bdsh[:, kc, sl],
                            start=False, stop=True, skip_group_check=True)
                    probs = atp.tile([128, 2, 272], BF16, tag="probs")
                    nc.scalar.activation(probs[:], psS[:, :, 0:272], AF.Exp)
                    for h2 in range(2):
                        nc.tensor.matmul(
                            psA[:, h2, 0:272], v65[:, kc, h, :],
                            probs[:, h2, :],
                            start=(kc == 0), stop=(kc == 7),
                            skip_group_check=True)
                s64 = atn.tile([65, 2, 272], F32, tag="s65")
                nc.scalar.activation(s64[64:65, :, :], psA[64:65, :, 0:272],
                                     AF.Copy)
                row = atn.tile([1, 2, 272], F32, tag="row")
                nc.gpsimd.dma_start(out=row[:], in_=s64[64:65, :, :])
                rec = atn.tile([1, 2, 272], F32, tag="rec")
                scr = atn.tile([1, 2, 272], F32, tag="scr")
                nc.vector.reciprocal_approx_accurate(rec[:], row[:], scr[:])
                rb = atn.tile([64, 2, 272], F32, tag="rb")
                nc.gpsimd.partition_broadcast(rb[:], rec[:])
                nc.vector.tensor_mul(
                    o_h[hr:hr + 64, hc, :].rearrange("p (a w) -> p a w", a=2),
                    psA[0:64, :, 0:272], rb[:])

            def bdread(h):
                # XBAR-transposed read: [q, k]-contiguous rows -> [k, q] tile
                bdsh = shp.tile([128, 8, WQ], BF16, tag="sh")
                for kc in range(8):
                    src = bass.AP(tensor=bd_d[h], offset=127 + 128 * kc,
                                  ap=[[BRS - 1, WQ], [1, 128]])
                    nc.sync.dma_start_transpose(bdsh[:, kc, :], src)
                return bdsh

            emit_bd(0)
            bdsh_t = bdread(0)
            for h in range(H):
                if h + 1 < H:
                    emit_bd(h + 1)
                    nxt = bdread(h + 1)
                else:
                    nxt = None
                emit_scores(h, bdsh_t)
                bdsh_t = nxt

        # ---- out-projection + residual (all local) ----
        with tc.tile_pool(name="pso", bufs=2, space="PSUM") as psop, \
             tc.tile_pool(name="aot", bufs=1) as aot:
            wo_sb = aot.tile([128, 4, D], F32R, tag="wo")
            nc.sync.dma_start(out=wo_sb[:],
                              in_=wo_d[:].rearrange("c p w -> p c w"))
            bo_sb = aot.tile([128, 4], F32)
            nc.sync.dma_start(out=bo_sb[:], in_=bo_d[:])
            for f in range(4):
                pso = psop.tile([128, 2, 512], F32, tag="pso")
                for hc in range(4):
                    st, sp = (hc == 0), (hc == 3)
                    for h2 in range(2):
                        nc.tensor.matmul(pso[:, h2, 0:272],
                                         _r(wo_sb[:, hc, ts(f, 128)]),
                                         _r(o_h[:, hc, h2 * 272:
                                                (h2 + 1) * 272]),
                                         start=st, stop=sp,
                                         skip_group_check=True)
                for h2 in range(2):
                    nc.vector.scalar_tensor_tensor(
                        out=h2w[:, f, h2 * 272:(h2 + 1) * 272],
                        in0=pso[:, h2, 0:272],
                        scalar=bo_sb[:, f:f + 1],
                        in1=hfe_sb[:, f, ds(woff + h2 * 272, 272)].bitcast(F32),
                        op0=ALU.add, op1=ALU.add)
        attpool.__exit__(None, None, None)
        actpool.__exit__(None, None, None)

        # ================= conv module =================
        gg = ctx.enter_context(tc.tile_pool(name="gg", bufs=1))
        g1t = gg.tile([128, 16, D], F32R)
        g2t = gg.tile([128, 16, D], F32R)
        gb1 = gg.tile([128, 16], F32)
        gb2 = gg.tile([128, 4], F32)
        g12pool = tc.tile_pool(name="g12", bufs=1)
        g12 = g12pool.__enter__()
        w1c = g12.tile([128, 4, 1024], F32R)
        w2c = g12.tile([128, 4, D], F32R)
        dgt = g12.tile([128, 4, KCONV, 128], BF16)
        for c in range(4):
            nc.sync.dma_start(out=w1c[:, c, :], in_=pw1_d[c])
        for c in range(4):
            nc.sync.dma_start(out=dgt[:, c, :, :],
                              in_=dwdg_d[c].rearrange("j p w -> p j w"))
        nc.sync.dma_start(out=w2c[:],
                          in_=pw2_d[:].rearrange("c p w -> p c w"))
        nc.scalar.dma_start(out=gb1[:], in_=bg1_d[:])
        nc.scalar.dma_start(out=gb2[:], in_=bg2_d[:])
        for j in range(16):
            nc.scalar.dma_start(out=g1t[:, j, :], in_=wg1_d[j])
        for j in range(16):
            nc.scalar.dma_start(out=g2t[:, j, :], in_=wg2_d[j])
        with tc.tile_pool(name="stF", bufs=1) as stF, \
             tc.tile_pool(name="stFt", bufs=1) as stFt:
            bp1 = stF.tile([128, 8], F32)
            nc.sync.dma_start(out=bp1[:], in_=bpw1_d[:])
            cm = stF.tile([1, WQ], F32)
            nc.sync.dma_start(out=cm[:], in_=cmask_d[:])
            cmb = stF.tile([128, WQ], F32)
            nc.gpsimd.partition_broadcast(cmb[:], cm[:])
            glu = stF.tile([128, 4, WQ], BF16)
            with tc.tile_pool(name="stF1", bufs=1) as stF1, \
                 tc.tile_pool(name="cvp1", bufs=1, space="PSUM") as cps:
                xn3 = stF1.tile([128, 4, WQ], F32R)
                emit_ln(h2w[:], xn3[:], WQ, [(0, 272), (272, 272)], stFt)
                for m in range(4):
                    psa = cps.tile([128, 2, 512], F32, tag="psa")
                    psg = cps.tile([128, 2, 512], F32, tag="psg")
                    for half in range(2):
                        sl = slice(half * 272, (half + 1) * 272)
                        for c in range(4):
                            nc.tensor.matmul(psa[:, half, 0:272],
                                             _r(w1c[:, c, ts(m, 128)]),
                                             _r(xn3[:, c, sl]),
                                             start=(c == 0), stop=(c == 3),
                                             skip_group_check=True)
                        for c in range(4):
                            nc.tensor.matmul(psg[:, half, 0:272],
                                             _r(w1c[:, c, 512 + m * 128:
                                                 512 + (m + 1) * 128]),
                                             _r(xn3[:, c, sl]),
                                             start=(c == 0), stop=(c == 3),
                                             skip_group_check=True)
                    sg = stFt.tile([128, WQ], F32, tag="sg")
                    sgv = sg[:].rearrange("p (a w) -> p a w", a=2)
                    gluv = glu[:, m, :].rearrange("p (a w) -> p a w", a=2)
                    nc.scalar.activation(sgv, psg[:, :, 0:272], AF.Sigmoid,
                                         bias=bp1[:, 4 + m:5 + m])
                    nc.vector.scalar_tensor_tensor(
                        out=gluv, in0=psa[:, :, 0:272], scalar=bp1[:, m:m + 1],
                        in1=sgv, op0=ALU.add, op1=ALU.mult)
                    nc.vector.tensor_mul(glu[:, m, :], glu[:, m, :], cmb[:])
            # depthwise conv: 31 accumulating diagonal bf16 matmuls per chunk
            acc = stF.tile([128, 4, HT], F32)
            bnpk = stF.tile([128, 8], F32)
            with tc.tile_pool(name="dgp", bufs=2, space="PSUM") as dgp:
                for c in range(4):
                    psC = dgp.tile([128, HT], F32, tag="psC")
                    for j in range(KCONV):
                        nc.tensor.matmul(psC[:], dgt[:, c, j, :],
                                         glu[:, c, 1 + j:1 + j + HT],
                                         start=(j == 0), stop=(j == KCONV - 1))
                    nc.scalar.activation(acc[:, c, :], psC[:], AF.Copy)
                    bst_t = stFt.tile([128, 6], F32, tag="bst")
                    nc.vector.bn_stats(bst_t[:], acc[:, c, :])
                    mv = stFt.tile([128, 2], F32, tag="mv")
                    nc.vector.bn_aggr(mv[:], bst_t[:])
                    nc.vector.tensor_copy(bnpk[:, 2 * c:2 * c + 1], mv[:, 0:1])
                    nc.vector.scalar_tensor_tensor(
                        out=bnpk[:, 2 * c + 1:2 * c + 2], in0=mv[:, 0:1],
                        scalar=mv[:, 0:1], in1=mv[:, 1:2],
                        op0=ALU.mult, op1=ALU.add)
            nc.sync.dma_start(out=cc_bn_in[:], in_=bnpk[:])
            nc.gpsimd.collective_compute(
                "AllReduce", ALU.add, ins=[cc_bn_in[:]],
                outs=[cc_bn_out[:]], replica_groups=ALLG)
            bnar8 = stF.tile([128, 8], F32)
            nc.sync.dma_start(out=bnar8[:], in_=cc_bn_out[:])
            bng_sb = stF.tile([128, 4], F32)
            nc.sync.dma_start(out=bng_sb[:], in_=bng_d[:])
            bnb_sb = stF.tile([128, 4], F32)
            nc.sync.dma_start(out=bnb_sb[:], in_=bnb_d[:])
            bp2 = stF.tile([128, 4], F32)
            nc.sync.dma_start(out=bp2[:], in_=bpw2_d[:])
            ysl = stF.tile([128, 4, HT], F32R)
            for c in range(4):
                mg = stFt.tile([128, 1], F32, tag="mg")
                nc.scalar.activation(mg[:], bnar8[:, 2 * c:2 * c + 1], AF.Copy,
                                     scale=1.0 / NCORES)
                e2 = stFt.tile([128, 1], F32, tag="e2c")
                nc.scalar.activation(e2[:], bnar8[:, 2 * c + 1:2 * c + 2],
                                     AF.Copy, scale=1.0 / NCORES)
                vg = stFt.tile([128, 1], F32, tag="vg")
                nc.vector.tensor_mul(vg[:], mg[:], mg[:])
                nc.vector.tensor_sub(vg[:], e2[:], vg[:])
                sdc = stFt.tile([128, 1], F32, tag="sdc")
                nc.scalar.activation(sdc[:], vg[:], AF.Sqrt, bias=epsP[:])
                rs = stFt.tile([128, 1], F32, tag="rsc")
                nc.vector.reciprocal(rs[:], sdc[:])
                s1 = stFt.tile([128, 1], F32, tag="s1c")
                nc.vector.tensor_mul(s1[:], rs[:], bng_sb[:, c:c + 1])
                s2 = stFt.tile([128, 1], F32, tag="s2c")
                nc.vector.tensor_mul(s2[:], mg[:], s1[:])
                nc.vector.tensor_sub(s2[:], bnb_sb[:, c:c + 1], s2[:])
                sg2 = stFt.tile([128, HT], F32, tag="sg2")
                nc.scalar.activation(sg2[:], acc[:, c, :], AF.Sigmoid,
                                     scale=s1[:], bias=s2[:])
                nc.vector.tensor_scalar(ysl[:, c, :], acc[:, c, :],
                                        s1[:], s2[:], ALU.mult, ALU.add)
                nc.vector.tensor_mul(ysl[:, c, :], ysl[:, c, :], sg2[:])
            with tc.tile_pool(name="cvp2", bufs=2, space="PSUM") as cps2:
                for f in range(4):
                    psw = cps2.tile([128, HT], F32, tag="psw")
                    for c in range(4):
                        nc.tensor.matmul(psw[:], _r(w2c[:, c, ts(f, 128)]),
                                         _r(ysl[:, c, :]),
                                         start=(c == 0), stop=(c == 3))
                    nc.vector.scalar_tensor_tensor(
                        out=h3_sb[:, f, :], in0=psw[:], scalar=bp2[:, f:f + 1],
                        in1=h2w[:, f, 16:16 + HT], op0=ALU.add, op1=ALU.add)
        g12pool.__exit__(None, None, None)

        # ================= FFN2 =================
        with tc.tile_pool(name="stG", bufs=1) as stG, \
             tc.tile_pool(name="stGt", bufs=1) as stGt:
            xn4 = stG.tile([128, 4, HT], F32R)
            emit_ln(h3_sb[:], xn4[:], HT, [(0, HT)], stGt)
            emit_ffn(xn4[:], h3_sb[:], h4_sb[:], g1t, gb1[:], g2t, gb2[:],
                     "f2")

        # ================= LN5 + output =================
        with tc.tile_pool(name="stH", bufs=1) as stH, \
             tc.tile_pool(name="stHt", bufs=2) as stHt:
            g5_sb = stH.tile([128, 4], F32)
            nc.sync.dma_start(out=g5_sb[:], in_=g5_d[:])
            b5_sb = stH.tile([128, 4], F32)
            nc.sync.dma_start(out=b5_sb[:], in_=b5_d[:])
            xn5 = stH.tile([128, 4, HT], F32)
            emit_ln(h4_sb[:], xn5[:], HT, [(0, HT)], stHt)
            for c in range(4):
                nc.vector.tensor_scalar(xn5[:, c, :], xn5[:, c, :],
                                        g5_sb[:, c:c + 1], b5_sb[:, c:c + 1],
                                        ALU.mult, ALU.add)
                nc.sync.dma_start(out=out_d[c], in_=xn5[:, c, :])
    return nc


_CACHE = {}


def build_nc():
    if "nc" not in _CACHE:
        nc = bacc.Bacc("TRN2", target_bir_lowering=False, debug=False,
                       num_devices=NCORES)
        _emit(nc)
        nc.compile()
        _CACHE["nc"] = nc
    return _CACHE["nc"]


def _chunk_cf(a2d):
    """[Dany, W] -> [Dany//128, 128, W] chunk-major channels-first."""
    d, w = a2d.shape
    return np.ascontiguousarray(a2d.reshape(d // 128, 128, w), dtype=np.float32)


def round_f32r(a):
    import ml_dtypes
    a = np.asarray(a, dtype=np.float32)
    hi = a.astype(ml_dtypes.bfloat16).astype(np.float32)
    lo = (a - hi).astype(ml_dtypes.bfloat16).astype(np.float32)
    return (hi + lo).astype(np.float32)


def _pcol(vec):
    """[Dout] per-channel vector -> [128, Dout//128] (partition, chunk)."""
    n = vec.shape[0]
    return np.ascontiguousarray(vec.reshape(n // 128, 128).T, dtype=np.float32)


def _jslice(w):
    """[D, DFF] -> [16, 128, 512] j-major stationary slices."""
    return np.ascontiguousarray(
        w.reshape(4, 128, 16, 128).transpose(2, 1, 0, 3).reshape(16, 128, 512),
        dtype=np.float32)


def make_in_maps(inputs):
    import ml_dtypes
    inputs = {k: np.asarray(v, dtype=np.float32) for k, v in inputs.items()}
    x = inputs["x"]; pos_emb = inputs["pos_emb"]
    ln1_g, ln1_b = inputs["ln1_g"], inputs["ln1_b"]
    ln2_g, ln2_b = inputs["ln2_g"], inputs["ln2_b"]
    ln3_g, ln3_b = inputs["ln3_g"], inputs["ln3_b"]
    ln4_g, ln4_b = inputs["ln4_g"], inputs["ln4_b"]
    ln5_g, ln5_b = inputs["ln5_g"], inputs["ln5_b"]

    w1f = ln1_g[:, None] * inputs["ff1_w1"]
    b1f = inputs["ff1_b1"] + ln1_b @ inputs["ff1_w1"]
    w2f = 0.5 * inputs["ff1_w2"]; b2f = 0.5 * inputs["ff1_b2"]
    wg1f = ln4_g[:, None] * inputs["ff2_w1"]
    bg1f = inputs["ff2_b1"] + ln4_b @ inputs["ff2_w1"]
    wg2f = 0.5 * inputs["ff2_w2"]; bg2f = 0.5 * inputs["ff2_b2"]

    s = DK ** -0.5
    pos_u_f = inputs["pos_u"].reshape(D); pos_v_f = inputs["pos_v"].reshape(D)
    wqf = s * (ln2_g[:, None] * inputs["wq"])
    bqf = s * (inputs["bq"] + ln2_b @ inputs["wq"] + pos_u_f)
    dqvf = s * (pos_v_f - pos_u_f)
    wkf = ln2_g[:, None] * inputs["wk"]
    bkf = inputs["bk"] + ln2_b @ inputs["wk"]
    wvf = ln2_g[:, None] * inputs["wv"]
    bvf = inputs["bv"] + ln2_b @ inputs["wv"]

    # positional frame, padded: 16 left + 2047 + 49 right
    p_pad = np.zeros((D, 2112), dtype=np.float32)
    p_pad[:, 16:16 + 2 * T - 1] = pos_emb[0].T

    pw1f = (inputs["pw1_w"] * ln3_g[None, :]).T            # [512, 1024]
    bpw1f = inputs["pw1_b"] + inputs["pw1_w"] @ ln3_b      # [1024]
    dwwf = inputs["dw_w"][:, 0, :]                         # [512, 31]
    dwdiag = np.zeros((4, KCONV, 128, 128), dtype=np.float32)
    ar = np.arange(128)
    for c4 in range(4):
        for j4 in range(KCONV):
            dwdiag[c4, j4, ar, ar] = dwwf[c4 * 128:(c4 + 1) * 128, j4]
    pw2f = inputs["pw2_w"].T                               # [512, 512]

    base = {
        "wf1": round_f32r(_jslice(w1f)), "bf1": _pcol(b1f),
        "wf2": round_f32r(_chunk_cf(w2f)), "bf2": _pcol(b2f),
        "wg1": round_f32r(_jslice(wg1f)), "bg1": _pcol(bg1f),
        "wg2": round_f32r(_chunk_cf(wg2f)), "bg2": _pcol(bg2f),
        "wq": round_f32r(_chunk_cf(wqf)), "bq": _pcol(bqf),
        "dqv": _pcol(dqvf),
        "wk": round_f32r(_chunk_cf(wkf)), "bk": _pcol(bkf),
        "wv": round_f32r(_chunk_cf(wvf)),
        "bvrow": np.ascontiguousarray(bvf.reshape(1, D), dtype=np.float32),
        "wp": round_f32r(_chunk_cf(inputs["wp"])),
        "wo": round_f32r(np.ascontiguousarray(
            inputs["wo"].reshape(4, 128, D), dtype=np.float32)),
        "bo": _pcol(inputs["bo"]),
        "pw1": round_f32r(_chunk_cf(pw1f)), "bpw1": _pcol(bpw1f),
        "dwdiag": dwdiag.astype(ml_dtypes.bfloat16),
        "bng": _pcol(inputs["bn_g"]), "bnb": _pcol(inputs["bn_b"]),
        "pw2": round_f32r(_chunk_cf(pw2f)), "bpw2": _pcol(inputs["pw2_b"]),
        "g5": _pcol(ln5_g), "b5": _pcol(ln5_b),
        "onc": np.ones((128, 1), dtype=np.float32),
        "idb": np.eye(128, dtype=np.float32).astype(ml_dtypes.bfloat16),
        "onv": np.ones((128, 64), dtype=ml_dtypes.bfloat16),
    }

    in_maps = []
    for c in range(NCORES):
        b, sc = c // 2, c % 2
        m = dict(base)
        xb = x[b, sc * HT:(sc + 1) * HT, :].T               # [512, 512]
        m["x_loc"] = round_f32r(_chunk_cf(xb))
        # per-core pre-shifted positional slice (band bases become static)
        lo = 512 - 512 * sc
        m["posT"] = round_f32r(_chunk_cf(p_pad[:, lo:lo + PW]))
        cmask = np.ones((1, WQ), dtype=np.float32)
        if sc == 0:
            cmask[0, :16] = 0.0
        else:
            cmask[0, WQ - 16:] = 0.0
        m["cmask"] = cmask
        in_maps.append(m)
    return in_maps


def assemble_out(results):
    out = np.empty((B, T, D), dtype=np.float32)
    for c in range(NCORES):
        b, sc = c // 2, c % 2
        ol = np.asarray(results[c]["out_loc"])              # [4, 128, 512]
        out[b, sc * HT:(sc + 1) * HT, :] = ol.reshape(D, HT).T
    return out


def kernel(**inputs):
    in_maps = make_in_maps(inputs)
    nc = build_nc()
    res = run_bass_kernel_spmd(nc, in_maps, list(range(NCORES)))
    return assemble_out(res.results)


# revision 55
# speedup vs baseline: 1.0391x; 1.0391x over previous
"""Conformer layer on 8 Trainium2 NeuronCores.

Sharding: core c handles batch b=c//2, token half sc=c%2 (512 tokens).
All stages are token-parallel; attention computes ALL 8 heads for the
core's own 544-token window (512 own + 16-token halos), so no collective
is needed after attention: the conv-module halo comes straight from the
locally computed window. Cross-core traffic: one pair AllGather of the
post-FFN1 hidden state and four tiny 8-way AllReduces of per-chunk
BatchNorm statistics.

Activations are channels-first ([channel, token]); matmuls run in
float32r (full PE rate at moving >= 256). The rel-position term bd is a
banded matmul bounced through DRAM in bf16 with a row-stride mismatch
that realizes rel-shift+transpose as one strided read; it is accumulated
onto the score PSUM by a bf16 identity matmul. The depthwise conv runs
as 31 accumulating diagonal bf16 matmuls per channel chunk. FFN weights
stream j-sliced so matmuls start before the full weight load completes.
"""

import numpy as np

import concourse.bass as bass
import concourse.mybir as mybir
import concourse.tile as tile
from concourse import bacc
from concourse.bass import ds, ts
from concourse.bass_utils import run_bass_kernel_spmd
from contextlib import ExitStack

F32 = mybir.dt.float32
F32R = mybir.dt.float32r
BF16 = mybir.dt.bfloat16
AF = mybir.ActivationFunctionType
ALU = mybir.AluOpType

D, DFF, H, DK, KCONV = 512, 2048, 8, 64, 31
B, T = 4, 1024
EPS = 1e-5
HT = 512            # own tokens per core
WQ = 544            # attention/conv window: 16 + 512 + 16
FR = 1056           # padded frame: 16 + 1024 + 16
PW = 1568           # local (pre-shifted) positional width
BRS = 1280          # bd dram row stride (elements)
BDSZ = 5 * 128 * BRS
TAILOFF = 4 * 128 * (BRS - 1) + 96
NCORES = 8

PAIRS = [[0, 1], [2, 3], [4, 5], [6, 7]]
ALLG = [[0, 1, 2, 3, 4, 5, 6, 7]]

# bd q-chunks: (q0, qw, band_base, band_pi_width)
BDCH = [(0, 128, 416, 384), (128, 128, 288, 384), (256, 128, 160, 384),
        (384, 128, 32, 384), (512, 32, 0, 352)]


def _r(ap):
    return ap.bitcast(F32R)


def _emit(nc):
    def inp(name, shape, dt=F32):
        return nc.dram_tensor(name, list(shape), dt, kind="ExternalInput")

    x_d = inp("x_loc", (4, 128, HT), F32R)
    wf1_d = inp("wf1", (16, 128, D), BF16); bf1_d = inp("bf1", (128, 16))
    wf2_d = inp("wf2", (16, 128, D), BF16); bf2_d = inp("bf2", (128, 4))
    wq_d = inp("wq", (4, 128, D), BF16); bq_d = inp("bq", (128, 4))
    dqv_d = inp("dqv", (128, 4))
    wk_d = inp("wk", (4, 128, D), BF16); bk_d = inp("bk", (128, 4))
    wv_d = inp("wv", (4, 128, D), BF16); bv_d = inp("bvrow", (1, D))
    wp_d = inp("wp", (4, 128, D), BF16)
    wo_d = inp("wo", (4, 128, D), F32R); bo_d = inp("bo", (128, 4))
    posT_d = inp("posT", (4, 128, PW), BF16)
    pw1_d = inp("pw1", (4, 128, 1024), BF16); bpw1_d = inp("bpw1", (128, 8))
    dwdg_d = inp("dwdiag", (4, KCONV, 128, 128), BF16)
    bng_d = inp("bng", (128, 4)); bnb_d = inp("bnb", (128, 4))
    pw2_d = inp("pw2", (4, 128, D), BF16); bpw2_d = inp("bpw2", (128, 4))
    cmask_d = inp("cmask", (1, WQ))
    wg1_d = inp("wg1", (16, 128, D), BF16); bg1_d = inp("bg1", (128, 16))
    wg2_d = inp("wg2", (16, 128, D), BF16); bg2_d = inp("bg2", (128, 4))
    g5_d = inp("g5", (128, 4)); b5_d = inp("b5", (128, 4))
    onc_d = inp("onc", (128, 1), F32R)
    idb_d = inp("idb", (128, 128), BF16)
    onv_d = inp("onv", (128, 64), BF16)

    out_d = nc.dram_tensor("out_loc", [4, 128, HT], F32, kind="ExternalOutput")

    cc_h_in = nc.dram_tensor("cc_h_in", [4, 128, HT], F32R)
    cc_h_out = nc.dram_tensor("cc_h_out", [8, 128, HT], F32R)
    cc_bn_in = nc.dram_tensor("cc_bn_in", [128, 8], F32)
    cc_bn_out = nc.dram_tensor("cc_bn_out", [128, 8], F32)
    bd_d = [nc.dram_tensor(f"bd_{h}", [BDSZ], BF16) for h in range(H)]

    ln_uid = [0]

    with tile.TileContext(nc) as tc, ExitStack() as ctx:
        const = ctx.enter_context(tc.tile_pool(name="const", bufs=1))
        ones_c = const.tile([128, 1], F32R)
        nc.sync.dma_start(out=ones_c[:], in_=onc_d[:])
        eps1 = const.tile([1, 1], F32); nc.vector.memset(eps1[:], EPS)
        epsP = const.tile([128, 1], F32); nc.vector.memset(epsP[:], EPS)
        identb = const.tile([128, 128], BF16)
        nc.sync.dma_start(out=identb[:], in_=idb_d[:])

        late = ctx.enter_context(tc.tile_pool(name="late", bufs=1))
        h2w = late.tile([128, 4, WQ], F32R)   # post-attention hidden window
        h3_sb = late.tile([128, 4, HT], F32R)    # post-conv hidden

        # conv weights: tiles allocated up-front, DMAs issued mid-attention
        g12pool = tc.tile_pool(name="g12", bufs=1)
        g12 = g12pool.__enter__()
        w1c = g12.tile([128, 4, 1024], BF16)
        w2c = g12.tile([128, 4, D], BF16)
        dgt = g12.tile([128, 4, KCONV, 128], BF16)

        actpool = tc.tile_pool(name="act", bufs=1)
        act = actpool.__enter__()
        # full padded hidden frame (post-FFN1), 16 zero tokens on both ends
        hfe_sb = act.tile([128, 4, FR], F32R)
        nc.vector.memset(hfe_sb[:, :, 0:16].bitcast(F32), 0.0)
        nc.vector.memset(hfe_sb[:, :, 16 + T:].bitcast(F32), 0.0)

        pid = nc.vector.partition_id()
        woff = (pid % 2) * 512      # window start in frame coords

        # ---------- LN helper (channels-first) ----------
        def emit_ln(x4, out4, W, blocks, sbp, post=None):
            ln_uid[0] += 1
            with tc.tile_pool(name=f"lnps{ln_uid[0]}", bufs=1,
                              space="PSUM") as lnps:
                for b0, bw in blocks:
                    x2 = sbp.tile([128, 4, bw], F32R, tag="ln_sq")
                    for c in range(4):
                        nc.vector.tensor_mul(x2[:, c, :],
                                             x4[:, c, b0:b0 + bw],
                                             x4[:, c, b0:b0 + bw])
                    pss = lnps.tile([1, bw], F32, tag="lns")
                    psq = lnps.tile([1, bw], F32, tag="lnq")
                    for c in range(4):
                        nc.tensor.matmul(pss[:], _r(ones_c[:]),
                                         _r(x4[:, c, b0:b0 + bw]),
                                         start=(c == 0), stop=(c == 3))
                    for c in range(4):
                        nc.tensor.matmul(psq[:], _r(ones_c[:]),
                                         _r(x2[:, c, :]),
                                         start=(c == 0), stop=(c == 3))
                    mng = sbp.tile([1, bw], F32, tag="ln_m")
                    nc.scalar.activation(mng[:], pss[:], AF.Copy,
                                         scale=-1.0 / D)
                    var = sbp.tile([1, bw], F32, tag="ln_var")
                    nc.scalar.activation(var[:], psq[:], AF.Copy, scale=1.0 / D)
                    m2 = sbp.tile([1, bw], F32, tag="ln_sd")
                    nc.vector.tensor_mul(m2[:], mng[:], mng[:])
                    nc.vector.tensor_sub(var[:], var[:], m2[:])
                    nc.scalar.activation(m2[:], var[:], AF.Sqrt, bias=eps1[:])
                    rec = sbp.tile([1, bw], F32, tag="ln_rs")
                    nc.vector.reciprocal(rec[:], m2[:])
                    nc.vector.tensor_mul(mng[:], mng[:], rec[:])
                    rb_t = sbp.tile([128, bw], F32, tag="ln_rb")
                    nc.gpsimd.partition_broadcast(rb_t[:], rec[:])
                    nb_t = sbp.tile([128, bw], F32, tag="ln_nb")
                    nc.gpsimd.partition_broadcast(nb_t[:], mng[:])
                    for c in range(4):
                        o = out4[:, c, b0:b0 + bw]
                        nc.vector.tensor_mul(o, x4[:, c, b0:b0 + bw], rb_t[:])
                        nc.vector.tensor_add(o, o, nb_t[:])
                        if post is not None:
                            post(c, b0, bw)

        # ---------- FFN helper (weights already in SBUF, j-sliced) ----------
        def emit_ffn(xn, xres, out, w1, b1, w2, b2, pref):
            # software-pipelined: psy[j+1] is emitted before psh[j] so the
            # in-order PE stream never waits on the silu of the current j
            with tc.tile_pool(name=pref + "t", bufs=3) as tp, \
                 tc.tile_pool(name=pref + "ps", bufs=2, space="PSUM") as psp, \
                 tc.tile_pool(name=pref + "ph", bufs=1, space="PSUM") as php:
                psh = php.tile([128, 4, HT], F32)
                y1s = [None] * 16

                def psy_stage(j):
                    psy = psp.tile([128, HT], F32, tag="psy")
                    for c in range(4):
                        nc.tensor.matmul(psy[:], w1[:, j, ts(c, 128)],
                                         xn[:, c, :],
                                         start=(c == 0), stop=(c == 3))
                    y1 = tp.tile([128, HT], BF16, tag="y1")
                    nc.scalar.activation(y1[:], psy[:], AF.Silu,
                                         bias=b1[:, j:j + 1])
                    y1s[j] = y1

                psy_stage(0)
                for j in range(16):
                    if j + 1 < 16:
                        psy_stage(j + 1)
                    for f in range(4):
                        nc.tensor.matmul(psh[:, f, :],
                                         w2[:, j, ts(f, 128)], y1s[j][:],
                                         start=(j == 0), stop=(j == 15))
                for c in range(4):
                    nc.vector.scalar_tensor_tensor(
                        out=out[:, c, :], in0=psh[:, c, :],
                        scalar=b2[:, c:c + 1], in1=xres[:, c, :],
                        op0=ALU.add, op1=ALU.add)

        # ================= Stage A: FFN1 on local tokens =================
        with tc.tile_pool(name="f1w", bufs=1) as f1w, \
             tc.tile_pool(name="stA", bufs=1) as stA, \
             tc.tile_pool(name="stAt", bufs=1) as stAt:
            x_sb = stA.tile([128, 4, HT], F32R)
            nc.sync.dma_start(out=x_sb[:],
                              in_=x_d[:].rearrange("c p w -> p c w"))
            w1 = f1w.tile([128, 16, D], BF16)
            w2 = f1w.tile([128, 16, D], BF16)
            for g in range(4):
                nc.sync.dma_start(
                    out=w1[:, 4 * g:4 * g + 4, :],
                    in_=wf1_d[4 * g:4 * g + 4].rearrange("j p w -> p j w"))
                nc.scalar.dma_start(
                    out=w2[:, 4 * g:4 * g + 4, :],
                    in_=wf2_d[4 * g:4 * g + 4].rearrange("j p w -> p j w"))
            b1 = f1w.tile([128, 16], F32)
            nc.sync.dma_start(out=b1[:], in_=bf1_d[:])
            b2 = f1w.tile([128, 4], F32)
            nc.sync.dma_start(out=b2[:], in_=bf2_d[:])
            h_sb = stA.tile([128, 4, HT], F32R)
            xn1 = stA.tile([128, 4, HT], BF16)
            emit_ln(x_sb[:], xn1[:], HT, [(0, HT)], stAt)
            emit_ffn(xn1[:], x_sb[:], h_sb[:], w1, b1[:], w2, b2[:], "f1")
            nc.gpsimd.dma_start(
                out=cc_h_in[:].rearrange("c p w -> p c w"), in_=h_sb[:])

        nc.gpsimd.collective_compute(
            "AllGather", ALU.bypass, ins=[cc_h_in[:]], outs=[cc_h_out[:]],
            replica_groups=PAIRS)

        # ============ attention (all 8 heads, own 544-token window) =======
        attpool = tc.tile_pool(name="attp", bufs=1)
        attp = attpool.__enter__()
        p_sb = attp.tile([128, 4, PW], BF16)
        # ---- p-projection (independent of AllGather; overlaps it) ----
        with tc.tile_pool(name="ppp", bufs=2, space="PSUM") as ppp, \
             tc.tile_pool(name="atw", bufs=1) as atw:
            wp_sb = atw.tile([128, 4, D], BF16, tag="wp")
            nc.sync.dma_start(out=wp_sb[:],
                              in_=wp_d[:].rearrange("c p w -> p c w"))
            pos_sb = atw.tile([128, 4, PW], BF16, tag="pos")
            nc.scalar.dma_start(out=pos_sb[:],
                                in_=posT_d[:].rearrange("c p w -> p c w"))
            for hp in range(4):
                psp_t = ppp.tile([128, 3, 512], F32, tag="psp")
                psp_u = ppp.tile([128, 32], F32, tag="pspu")
                for c in range(4):
                    st, sp = (c == 0), (c == 3)
                    for pc in range(3):
                        nc.tensor.matmul(
                            psp_t[:, pc, :], wp_sb[:, c, ts(hp, 128)],
                            pos_sb[:, c, ts(pc, 512)], start=st, stop=sp)
                    nc.tensor.matmul(
                        psp_u[:], wp_sb[:, c, ts(hp, 128)],
                        pos_sb[:, c, 1536:PW], start=st, stop=sp)
                for pc in range(3):
                    if hp % 2 == 0:
                        nc.vector.tensor_copy(
                            p_sb[:, hp, ts(pc, 512)],
                            psp_t[:, pc, :])
                    else:
                        nc.scalar.activation(
                            p_sb[:, hp, ts(pc, 512)],
                            psp_t[:, pc, :], AF.Copy)
                nc.vector.tensor_copy(p_sb[:, hp, 1536:PW],
                                      psp_u[:])

        # ---- gather h_full into frame, LN2 over padded frame ----
        for half in range(2):
            nc.gpsimd.dma_start(
                out=hfe_sb[:, :, 16 + half * HT:16 + (half + 1) * HT],
                in_=cc_h_out[half * 4:half * 4 + 4].rearrange(
                    "c p w -> p c w"))
        v65pool = tc.tile_pool(name="v65p", bufs=1)
        v65pl = v65pool.__enter__()
        v65 = v65pl.tile([128, 8, 8, 65], BF16)
        xn2pool = tc.tile_pool(name="xn2", bufs=1)
        xn2pl = xn2pool.__enter__()
        xn2p = xn2pl.tile([128, 4, FR], BF16)
        with tc.tile_pool(name="stC", bufs=1) as stC, \
             tc.tile_pool(name="stCt", bufs=2) as stCt:
            emit_ln(hfe_sb[:], xn2p[:], FR,
                    [(0, 352), (352, 352), (704, 352)], stCt)

            # window copy for q (dynamic slice -> static tile)
            xn2w = stC.tile([128, 4, WQ], BF16)
            for c in range(4):
                nc.vector.tensor_copy(xn2w[:, c, :],
                                      xn2p[:, c, ds(woff, WQ)])

            # ---- q, qv, k, v projections ----
            q_sb = attp.tile([128, 4, WQ], BF16)
            qv_sb = attp.tile([128, 4, WQ], BF16)
            k_sb = attp.tile([128, 4, T], BF16)
            nc.sync.dma_start(out=v65[:, :, :, 64:65],
                              in_=onv_d[:].rearrange("p (a b) -> p a b", a=8))
            with tc.tile_pool(name="qkp", bufs=2, space="PSUM") as psp:
                wq_sb = stC.tile([128, 4, D], BF16)
                wk_sb = stC.tile([128, 4, D], BF16)
                wv_sb = xn2pl.tile([128, 4, D], BF16)
                nc.sync.dma_start(out=wq_sb[:],
                                  in_=wq_d[:].rearrange("c p w -> p c w"))
                nc.scalar.dma_start(out=wk_sb[:],
                                    in_=wk_d[:].rearrange("c p w -> p c w"))
                nc.sync.dma_start(out=wv_sb[:],
                                  in_=wv_d[:].rearrange("c p w -> p c w"))
                bq_sb = stC.tile([128, 4], F32)
                nc.sync.dma_start(out=bq_sb[:], in_=bq_d[:])
                bk_sb = stC.tile([128, 4], F32)
                nc.sync.dma_start(out=bk_sb[:], in_=bk_d[:])
                dqv_sb = stC.tile([128, 4], F32)
                nc.sync.dma_start(out=dqv_sb[:], in_=dqv_d[:])
                bv_sb = stC.tile([1, D], F32)
                nc.sync.dma_start(out=bv_sb[:], in_=bv_d[:])
                bvb_sb = stC.tile([128, D], F32)
                nc.gpsimd.partition_broadcast(bvb_sb[:], bv_sb[:])
                for hp in range(4):
                    psq = psp.tile([128, 2, 512], F32, tag="psq")
                    for c in range(4):
                        st, sp = (c == 0), (c == 3)
                        for h2 in range(2):
                            nc.tensor.matmul(psq[:, h2, 0:272],
                                             wq_sb[:, c, ts(hp, 128)],
                                             xn2w[:, c, h2 * 272:
                                                  (h2 + 1) * 272],
                                             start=st, stop=sp,
                                             skip_group_check=True)
                    nc.vector.tensor_scalar_add(
                        q_sb[:, hp, :].rearrange("p (a w) -> p a w", a=2),
                        psq[:, :, 0:272], bq_sb[:, hp:hp + 1])
                    nc.vector.tensor_scalar_add(qv_sb[:, hp, :],
                                                q_sb[:, hp, :],
                                                dqv_sb[:, hp:hp + 1])
                    for th in range(2):
                        psk = psp.tile([128, 512], F32, tag="psk")
                        for c in range(4):
                            nc.tensor.matmul(
                                psk[:], wk_sb[:, c, ts(hp, 128)],
                                xn2p[:, c, 16 + th * 512:16 + th * 512 + 512],
                                start=(c == 0), stop=(c == 3))
                        nc.vector.tensor_scalar_add(
                            k_sb[:, hp, th * 512:(th + 1) * 512], psk[:],
                            bk_sb[:, hp:hp + 1])
                for tq in range(8):
                    psv = psp.tile([128, 512], F32, tag="psv")
                    for c in range(4):
                        nc.tensor.matmul(
                            psv[:], xn2p[:, c, 16 + tq * 128:16 + (tq + 1) * 128],
                            wv_sb[:, c, :],
                            start=(c == 0), stop=(c == 3))
                    nc.vector.tensor_add(
                        v65[:, tq, :, 0:64],
                        psv[:].rearrange("p (h d) -> p h d", h=8),
                        bvb_sb[:].rearrange("p (h d) -> p h d", h=8))

        xn2pool.__exit__(None, None, None)

        # ---- per-head: banded bd -> DRAM bounce -> scores -> av ----
        o_h = attp.tile([128, 4, WQ], F32R)
        with tc.tile_pool(name="bds", bufs=2) as bdp, \
             tc.tile_pool(name="bdsh", bufs=2) as shp, \
             tc.tile_pool(name="atp", bufs=2) as atp, \
             tc.tile_pool(name="atn", bufs=1) as atn, \
             tc.tile_pool(name="psb", bufs=2, space="PSUM") as psb, \
             tc.tile_pool(name="pss", bufs=2, space="PSUM") as pss, \
             tc.tile_pool(name="psav", bufs=1, space="PSUM") as psav:

            def emit_bd(h):
                hc, hr = h // 2, 64 * (h % 2)
                for ci, (q0, qw, m0, pw) in enumerate(BDCH):
                    bdw = bdp.tile([128, 3, 384], BF16, tag="bdw")
                    for pi in range(3):
                        psB = psb.tile([128, 384], F32, tag="psB")
                        nc.tensor.matmul(
                            psB[0:qw, 0:pw],
                            qv_sb[hr:hr + 64, hc, q0:q0 + qw],
                            p_sb[hr:hr + 64, hc,
                                 m0 + pi * pw:m0 + (pi + 1) * pw],
                            start=True, stop=True)
                        if pi != 1:
                            nc.vector.tensor_copy(bdw[0:qw, pi, 0:pw],
                                                  psB[0:qw, 0:pw])
                        else:
                            nc.scalar.activation(bdw[0:qw, pi, 0:pw],
                                                 psB[0:qw, 0:pw], AF.Copy)
                    off = ci * 128 * (BRS - 1) if qw == 128 else TAILOFF
                    dst = bass.AP(tensor=bd_d[h], offset=off,
                                  ap=[[BRS, qw], [1, 3 * pw]])
                    nc.gpsimd.dma_start(
                        out=dst, in_=bdw[0:qw, :, 0:pw])

            def emit_scores(h, bdsh):
                hc, hr = h // 2, 64 * (h % 2)
                psA = psav.tile([65, 2, 512], F32, tag="psAV")
                probss = [None] * 8

                def sc_stage(kc):
                    psS = pss.tile([128, 2, 512], F32, tag="psS")
                    for h2 in range(2):
                        sl = slice(h2 * 272, (h2 + 1) * 272)
                        nc.tensor.matmul(
                            psS[:, h2, 0:272],
                            k_sb[hr:hr + 64, hc, ts(kc, 128)],
                            q_sb[hr:hr + 64, hc, sl],
                            start=True, stop=False, skip_group_check=True)
                        nc.tensor.matmul(
                            psS[:, h2, 0:272], identb[:], bdsh[:, kc, sl],
                            start=False, stop=True, skip_group_check=True)
                    probs = atp.tile([128, 2, 272], BF16, tag="probs")
                    nc.scalar.activation(probs[:], psS[:, :, 0:272], AF.Exp)
                    probss[kc] = probs

                sc_stage(0)
                for kc in range(8):
                    if kc + 1 < 8:
                        sc_stage(kc + 1)
                    for h2 in range(2):
                        nc.tensor.matmul(
                            psA[:, h2, 0:272], v65[:, kc, h, :],
                            probss[kc][:, h2, :],
                            start=(kc == 0), stop=(kc == 7),
                            skip_group_check=True)
                s64 = atn.tile([65, 2, 272], F32, tag="s65")
                nc.scalar.activation(s64[64:65, :, :], psA[64:65, :, 0:272],
                                     AF.Copy)
                row = atn.tile([1, 2, 272], F32, tag="row")
                nc.gpsimd.dma_start(out=row[:], in_=s64[64:65, :, :])
                rec = atn.tile([1, 2, 272], F32, tag="rec")
                scr = atn.tile([1, 2, 272], F32, tag="scr")
                nc.vector.reciprocal_approx_accurate(rec[:], row[:], scr[:])
                rb = atn.tile([64, 2, 272], F32, tag="rb")
                nc.gpsimd.partition_broadcast(rb[:], rec[:])
                nc.vector.tensor_mul(
                    o_h[hr:hr + 64, hc, :].rearrange("p (a w) -> p a w", a=2),
                    psA[0:64, :, 0:272], rb[:])

            def bdread(h):
                # XBAR-transposed read: [q, k]-contiguous rows -> [k, q] tile
                bdsh = shp.tile([128, 8, WQ], BF16, tag="sh")
                for kc in range(8):
                    src = bass.AP(tensor=bd_d[h], offset=127 + 128 * kc,
                                  ap=[[BRS - 1, WQ], [1, 128]])
                    nc.sync.dma_start_transpose(bdsh[:, kc, :], src)
                return bdsh

            emit_bd(0)
            bdsh_t = bdread(0)
            for h in range(H):
                if h + 1 < H:
                    emit_bd(h + 1)
                    nxt = bdread(h + 1)
                else:
                    nxt = None
                if h == 3:
                    nc.gpsimd.dma_start(
                        out=w1c[:], in_=pw1_d[:].rearrange("c p w -> p c w"))
                    nc.gpsimd.dma_start(
                        out=w2c[:], in_=pw2_d[:].rearrange("c p w -> p c w"))
                if h == 5:
                    nc.gpsimd.dma_start(
                        out=dgt[:].rearrange("p c j w -> p (c j) w"),
                        in_=dwdg_d[:].rearrange("c j p w -> p (c j) w"))
                emit_scores(h, bdsh_t)
                bdsh_t = nxt

        v65pool.__exit__(None, None, None)

        # ---- out-projection + residual (all local) ----
        with tc.tile_pool(name="pso", bufs=2, space="PSUM") as psop, \
             tc.tile_pool(name="aot", bufs=1) as aot:
            wo_sb = aot.tile([128, 4, D], F32R, tag="wo")
            nc.sync.dma_start(out=wo_sb[:],
                              in_=wo_d[:].rearrange("c p w -> p c w"))
            bo_sb = aot.tile([128, 4], F32)
            nc.sync.dma_start(out=bo_sb[:], in_=bo_d[:])
            for f in range(4):
                pso = psop.tile([128, 2, 512], F32, tag="pso")
                for hc in range(4):
                    st, sp = (hc == 0), (hc == 3)
                    for h2 in range(2):
                        nc.tensor.matmul(pso[:, h2, 0:272],
                                         _r(wo_sb[:, hc, ts(f, 128)]),
                                         _r(o_h[:, hc, h2 * 272:
                                                (h2 + 1) * 272]),
                                         start=st, stop=sp,
                                         skip_group_check=True)
                for h2 in range(2):
                    nc.vector.scalar_tensor_tensor(
                        out=h2w[:, f, h2 * 272:(h2 + 1) * 272],
                        in0=pso[:, h2, 0:272],
                        scalar=bo_sb[:, f:f + 1],
                        in1=hfe_sb[:, f, ds(woff + h2 * 272, 272)].bitcast(F32),
                        op0=ALU.add, op1=ALU.add)
        attpool.__exit__(None, None, None)
        actpool.__exit__(None, None, None)

        # ================= conv module =================
        ggpool = tc.tile_pool(name="gg", bufs=1)
        gg = ggpool.__enter__()
        h4_sb = gg.tile([128, 4, HT], F32R)    # post-FFN2 hidden
        g1t = gg.tile([128, 16, D], BF16)
        g2t = gg.tile([128, 16, D], BF16)
        gb1 = gg.tile([128, 16], F32)
        gb2 = gg.tile([128, 4], F32)
        nc.scalar.dma_start(out=gb1[:], in_=bg1_d[:])
        nc.scalar.dma_start(out=gb2[:], in_=bg2_d[:])
        for g in range(4):
            nc.scalar.dma_start(
                out=g1t[:, 4 * g:4 * g + 4, :],
                in_=wg1_d[4 * g:4 * g + 4].rearrange("j p w -> p j w"))
        for g in range(4):
            nc.scalar.dma_start(
                out=g2t[:, 4 * g:4 * g + 4, :],
                in_=wg2_d[4 * g:4 * g + 4].rearrange("j p w -> p j w"))
        with tc.tile_pool(name="stF", bufs=1) as stF, \
             tc.tile_pool(name="stFt", bufs=1) as stFt:
            bp1 = stF.tile([128, 8], F32)
            nc.sync.dma_start(out=bp1[:], in_=bpw1_d[:])
            cm = stF.tile([1, WQ], F32)
            nc.sync.dma_start(out=cm[:], in_=cmask_d[:])
            cmb = stF.tile([128, WQ], F32)
            nc.gpsimd.partition_broadcast(cmb[:], cm[:])
            glu = stF.tile([128, 4, WQ], BF16)
            with tc.tile_pool(name="stF1", bufs=1) as stF1, \
                 tc.tile_pool(name="cvp1", bufs=1, space="PSUM") as cps:
                xn3 = stF1.tile([128, 4, WQ], BF16)
                emit_ln(h2w[:], xn3[:], WQ, [(0, 272), (272, 272)], stFt)
                for m in range(4):
                    psa = cps.tile([128, 2, 512], F32, tag="psa")
                    psg = cps.tile([128, 2, 512], F32, tag="psg")
                    for half in range(2):
                        sl = slice(half * 272, (half + 1) * 272)
                        for c in range(4):
                            nc.tensor.matmul(psa[:, half, 0:272],
                                             w1c[:, c, ts(m, 128)],
                                             xn3[:, c, sl],
                                             start=(c == 0), stop=(c == 3),
                                             skip_group_check=True)
                        for c in range(4):
                            nc.tensor.matmul(psg[:, half, 0:272],
                                             w1c[:, c, 512 + m * 128:
                                                 512 + (m + 1) * 128],
                                             xn3[:, c, sl],
                                             start=(c == 0), stop=(c == 3),
                                             skip_group_check=True)
                    sg = stFt.tile([128, WQ], F32, tag="sg")
                    sgv = sg[:].rearrange("p (a w) -> p a w", a=2)
                    gluv = glu[:, m, :].rearrange("p (a w) -> p a w", a=2)
                    nc.scalar.activation(sgv, psg[:, :, 0:272], AF.Sigmoid,
                                         bias=bp1[:, 4 + m:5 + m])
                    nc.vector.scalar_tensor_tensor(
                        out=gluv, in0=psa[:, :, 0:272], scalar=bp1[:, m:m + 1],
                        in1=sgv, op0=ALU.add, op1=ALU.mult)
                    nc.vector.tensor_mul(glu[:, m, :], glu[:, m, :], cmb[:])
            # depthwise conv: 31 accumulating diagonal bf16 matmuls per chunk
            acc = stF.tile([128, 4, HT], F32)
            bnpk = stF.tile([128, 8], F32)
            with tc.tile_pool(name="dgp", bufs=2, space="PSUM") as dgp:
                for c in range(4):
                    psC = dgp.tile([128, HT], F32, tag="psC")
                    for j in range(KCONV):
                        nc.tensor.matmul(psC[:], dgt[:, c, j, :],
                                         glu[:, c, 1 + j:1 + j + HT],
                                         start=(j == 0), stop=(j == KCONV - 1))
                    nc.scalar.activation(acc[:, c, :], psC[:], AF.Copy)
                    bst_t = stFt.tile([128, 6], F32, tag="bst")
                    nc.vector.bn_stats(bst_t[:], acc[:, c, :])
                    mv = stFt.tile([128, 2], F32, tag="mv")
                    nc.vector.bn_aggr(mv[:], bst_t[:])
                    nc.vector.tensor_copy(bnpk[:, 2 * c:2 * c + 1], mv[:, 0:1])
                    nc.vector.scalar_tensor_tensor(
                        out=bnpk[:, 2 * c + 1:2 * c + 2], in0=mv[:, 0:1],
                        scalar=mv[:, 0:1], in1=mv[:, 1:2],
                        op0=ALU.mult, op1=ALU.add)
            nc.sync.dma_start(out=cc_bn_in[:], in_=bnpk[:])
            nc.gpsimd.collective_compute(
                "AllReduce", ALU.add, ins=[cc_bn_in[:]],
                outs=[cc_bn_out[:]], replica_groups=ALLG)
            bnar8 = stF.tile([128, 8], F32)
            nc.sync.dma_start(out=bnar8[:], in_=cc_bn_out[:])
            bng_sb = stF.tile([128, 4], F32)
            nc.sync.dma_start(out=bng_sb[:], in_=bng_d[:])
            bnb_sb = stF.tile([128, 4], F32)
            nc.sync.dma_start(out=bnb_sb[:], in_=bnb_d[:])
            bp2 = stF.tile([128, 4], F32)
            nc.sync.dma_start(out=bp2[:], in_=bpw2_d[:])
            ysl = stF.tile([128, 4, HT], BF16)
            for c in range(4):
                mg = stFt.tile([128, 1], F32, tag="mg")
                nc.scalar.activation(mg[:], bnar8[:, 2 * c:2 * c + 1], AF.Copy,
                                     scale=1.0 / NCORES)
                e2 = stFt.tile([128, 1], F32, tag="e2c")
                nc.scalar.activation(e2[:], bnar8[:, 2 * c + 1:2 * c + 2],
                                     AF.Copy, scale=1.0 / NCORES)
                vg = stFt.tile([128, 1], F32, tag="vg")
                nc.vector.tensor_mul(vg[:], mg[:], mg[:])
                nc.vector.tensor_sub(vg[:], e2[:], vg[:])
                sdc = stFt.tile([128, 1], F32, tag="sdc")
                nc.scalar.activation(sdc[:], vg[:], AF.Sqrt, bias=epsP[:])
                rs = stFt.tile([128, 1], F32, tag="rsc")
                nc.vector.reciprocal(rs[:], sdc[:])
                s1 = stFt.tile([128, 1], F32, tag="s1c")
                nc.vector.tensor_mul(s1[:], rs[:], bng_sb[:, c:c + 1])
                s2 = stFt.tile([128, 1], F32, tag="s2c")
                nc.vector.tensor_mul(s2[:], mg[:], s1[:])
                nc.vector.tensor_sub(s2[:], bnb_sb[:, c:c + 1], s2[:])
                sg2 = stFt.tile([128, HT], F32, tag="sg2")
                nc.scalar.activation(sg2[:], acc[:, c, :], AF.Sigmoid,
                                     scale=s1[:], bias=s2[:])
                nc.vector.tensor_scalar(ysl[:, c, :], acc[:, c, :],
                                        s1[:], s2[:], ALU.mult, ALU.add)
                nc.vector.tensor_mul(ysl[:, c, :], ysl[:, c, :], sg2[:])
            with tc.tile_pool(name="cvp2", bufs=2, space="PSUM") as cps2:
                for f in range(4):
                    psw = cps2.tile([128, HT], F32, tag="psw")
                    for c in range(4):
                        nc.tensor.matmul(psw[:], w2c[:, c, ts(f, 128)],
                                         ysl[:, c, :],
                                         start=(c == 0), stop=(c == 3))
                    nc.vector.scalar_tensor_tensor(
                        out=h3_sb[:, f, :], in0=psw[:], scalar=bp2[:, f:f + 1],
                        in1=h2w[:, f, 16:16 + HT], op0=ALU.add, op1=ALU.add)

        # ================= FFN2 =================
        with tc.tile_pool(name="stG", bufs=1) as stG, \
             tc.tile_pool(name="stGt", bufs=1) as stGt:
            xn4 = stG.tile([128, 4, HT], BF16)
            emit_ln(h3_sb[:], xn4[:], HT, [(0, HT)], stGt)
            emit_ffn(xn4[:], h3_sb[:], h4_sb[:], g1t, gb1[:], g2t, gb2[:],
                     "f2")

        # ================= LN5 + output =================
        with tc.tile_pool(name="stH", bufs=1) as stH, \
             tc.tile_pool(name="stHt", bufs=2) as stHt:
            g5_sb = stH.tile([128, 4], F32)
            nc.sync.dma_start(out=g5_sb[:], in_=g5_d[:])
            b5_sb = stH.tile([128, 4], F32)
            nc.sync.dma_start(out=b5_sb[:], in_=b5_d[:])
            xn5 = stH.tile([128, 4, HT], F32)
            emit_ln(h4_sb[:], xn5[:], HT, [(0, HT)], stHt)
            for c in range(4):
                nc.vector.tensor_scalar(xn5[:, c, :], xn5[:, c, :],
                                        g5_sb[:, c:c + 1], b5_sb[:, c:c + 1],
                                        ALU.mult, ALU.add)
                nc.sync.dma_start(out=out_d[c], in_=xn5[:, c, :])
        ggpool.__exit__(None, None, None)
        g12pool.__exit__(None, None, None)
    return nc


_CACHE = {}


def build_nc():
    if "nc" not in _CACHE:
        nc = bacc.Bacc("TRN2", target_bir_lowering=False, debug=False,
                       num_devices=NCORES)
        _emit(nc)
        nc.compile()
        _CACHE["nc"] = nc
    return _CACHE["nc"]


def _chunk_cf(a2d):
    """[Dany, W] -> [Dany//128, 128, W] chunk-major channels-first."""
    d, w = a2d.shape
    return np.ascontiguousarray(a2d.reshape(d // 128, 128, w), dtype=np.float32)


def round_f32r(a):
    import ml_dtypes
    a = np.asarray(a, dtype=np.float32)
    hi = a.astype(ml_dtypes.bfloat16).astype(np.float32)
    lo = (a - hi).astype(ml_dtypes.bfloat16).astype(np.float32)
    return (hi + lo).astype(np.float32)


def _pcol(vec):
    """[Dout] per-channel vector -> [128, Dout//128] (partition, chunk)."""
    n = vec.shape[0]
    return np.ascontiguousarray(vec.reshape(n // 128, 128).T, dtype=np.float32)


def _jslice(w):
    """[D, DFF] -> [16, 128, 512] j-major stationary slices."""
    return np.ascontiguousarray(
        w.reshape(4, 128, 16, 128).transpose(2, 1, 0, 3).reshape(16, 128, 512),
        dtype=np.float32)


def make_in_maps(inputs):
    import ml_dtypes
    inputs = {k: np.asarray(v, dtype=np.float32) for k, v in inputs.items()}
    x = inputs["x"]; pos_emb = inputs["pos_emb"]
    ln1_g, ln1_b = inputs["ln1_g"], inputs["ln1_b"]
    ln2_g, ln2_b = inputs["ln2_g"], inputs["ln2_b"]
    ln3_g, ln3_b = inputs["ln3_g"], inputs["ln3_b"]
    ln4_g, ln4_b = inputs["ln4_g"], inputs["ln4_b"]
    ln5_g, ln5_b = inputs["ln5_g"], inputs["ln5_b"]

    w1f = ln1_g[:, None] * inputs["ff1_w1"]
    b1f = inputs["ff1_b1"] + ln1_b @ inputs["ff1_w1"]
    w2f = 0.5 * inputs["ff1_w2"]; b2f = 0.5 * inputs["ff1_b2"]
    wg1f = ln4_g[:, None] * inputs["ff2_w1"]
    bg1f = inputs["ff2_b1"] + ln4_b @ inputs["ff2_w1"]
    wg2f = 0.5 * inputs["ff2_w2"]; bg2f = 0.5 * inputs["ff2_b2"]

    s = DK ** -0.5
    pos_u_f = inputs["pos_u"].reshape(D); pos_v_f = inputs["pos_v"].reshape(D)
    wqf = s * (ln2_g[:, None] * inputs["wq"])
    bqf = s * (inputs["bq"] + ln2_b @ inputs["wq"] + pos_u_f)
    dqvf = s * (pos_v_f - pos_u_f)
    wkf = ln2_g[:, None] * inputs["wk"]
    bkf = inputs["bk"] + ln2_b @ inputs["wk"]
    wvf = ln2_g[:, None] * inputs["wv"]
    bvf = inputs["bv"] + ln2_b @ inputs["wv"]

    # positional frame, padded: 16 left + 2047 + 49 right
    p_pad = np.zeros((D, 2112), dtype=np.float32)
    p_pad[:, 16:16 + 2 * T - 1] = pos_emb[0].T

    pw1f = (inputs["pw1_w"] * ln3_g[None, :]).T            # [512, 1024]
    bpw1f = inputs["pw1_b"] + inputs["pw1_w"] @ ln3_b      # [1024]
    dwwf = inputs["dw_w"][:, 0, :]                         # [512, 31]
    dwdiag = np.zeros((4, KCONV, 128, 128), dtype=np.float32)
    ar = np.arange(128)
    for c4 in range(4):
        for j4 in range(KCONV):
            dwdiag[c4, j4, ar, ar] = dwwf[c4 * 128:(c4 + 1) * 128, j4]
    pw2f = inputs["pw2_w"].T                               # [512, 512]

    bf = lambda a: a.astype(ml_dtypes.bfloat16)
    base = {
        "wf1": bf(_jslice(w1f)), "bf1": _pcol(b1f),
        "wf2": bf(_chunk_cf(w2f)), "bf2": _pcol(b2f),
        "wg1": bf(_jslice(wg1f)), "bg1": _pcol(bg1f),
        "wg2": bf(_chunk_cf(wg2f)), "bg2": _pcol(bg2f),
        "wq": bf(_chunk_cf(wqf)), "bq": _pcol(bqf),
        "dqv": _pcol(dqvf),
        "wk": bf(_chunk_cf(wkf)), "bk": _pcol(bkf),
        "wv": bf(_chunk_cf(wvf)),
        "bvrow": np.ascontiguousarray(bvf.reshape(1, D), dtype=np.float32),
        "wp": bf(_chunk_cf(inputs["wp"])),
        "wo": round_f32r(np.ascontiguousarray(
            inputs["wo"].reshape(4, 128, D), dtype=np.float32)),
        "bo": _pcol(inputs["bo"]),
        "pw1": bf(_chunk_cf(pw1f)), "bpw1": _pcol(bpw1f),
        "dwdiag": bf(dwdiag),
        "bng": _pcol(inputs["bn_g"]), "bnb": _pcol(inputs["bn_b"]),
        "pw2": bf(_chunk_cf(pw2f)), "bpw2": _pcol(inputs["pw2_b"]),
        "g5": _pcol(ln5_g), "b5": _pcol(ln5_b),
        "onc": np.ones((128, 1), dtype=np.float32),
        "idb": np.eye(128, dtype=np.float32).astype(ml_dtypes.bfloat16),
        "onv": np.ones((128, 64), dtype=ml_dtypes.bfloat16),
    }

    in_maps = []
    for c in range(NCORES):
        b, sc = c // 2, c % 2
        m = dict(base)
        xb = x[b, sc * HT:(sc + 1) * HT, :].T               # [512, 512]
        m["x_loc"] = round_f32r(_chunk_cf(xb))
        # per-core pre-shifted positional slice (band bases become static)
        lo = 512 - 512 * sc
        m["posT"] = bf(_chunk_cf(p_pad[:, lo:lo + PW]))
        cmask = np.ones((1, WQ), dtype=np.float32)
        if sc == 0:
            cmask[0, :16] = 0.0
        else:
            cmask[0, WQ - 16:] = 0.0
        m["cmask"] = cmask
        in_maps.append(m)
    return in_maps


def assemble_out(results):
    out = np.empty((B, T, D), dtype=np.float32)
    for c in range(NCORES):
        b, sc = c // 2, c % 2
        ol = np.asarray(results[c]["out_loc"])              # [4, 128, 512]
        out[b, sc * HT:(sc + 1) * HT, :] = ol.reshape(D, HT).T
    return out


def kernel(**inputs):
    in_maps = make_in_maps(inputs)
    nc = build_nc()
    res = run_bass_kernel_spmd(nc, in_maps, list(range(NCORES)))
    return assemble_out(res.results)


# revision 56
# speedup vs baseline: 1.1187x; 1.0766x over previous
"""Conformer layer on 8 Trainium2 NeuronCores.

Sharding: core c handles batch b=c//2, token half sc=c%2 (512 tokens).
All stages are token-parallel; attention computes ALL 8 heads for the
core's own 544-token window (512 own + 16-token halos), so no collective
is needed after attention: the conv-module halo comes straight from the
locally computed window. Cross-core traffic: one pair AllGather of the
post-FFN1 hidden state and four tiny 8-way AllReduces of per-chunk
BatchNorm statistics.

Activations are channels-first ([channel, token]); matmuls run in
float32r (full PE rate at moving >= 256). The rel-position term bd is a
banded matmul bounced through DRAM in bf16 with a row-stride mismatch
that realizes rel-shift+transpose as one strided read; it is accumulated
onto the score PSUM by a bf16 identity matmul. The depthwise conv runs
as 31 accumulating diagonal bf16 matmuls per channel chunk. FFN weights
stream j-sliced so matmuls start before the full weight load completes.
"""

import numpy as np

import concourse.bass as bass
import concourse.mybir as mybir
import concourse.tile as tile
from concourse import bacc
from concourse.bass import ds, ts
from concourse.bass_utils import run_bass_kernel_spmd
from contextlib import ExitStack

F32 = mybir.dt.float32
F32R = mybir.dt.float32r
BF16 = mybir.dt.bfloat16
AF = mybir.ActivationFunctionType
ALU = mybir.AluOpType

D, DFF, H, DK, KCONV = 512, 2048, 8, 64, 31
B, T = 4, 1024
EPS = 1e-5
HT = 512            # own tokens per core
WQ = 544            # attention/conv window: 16 + 512 + 16
FR = 1056           # padded frame: 16 + 1024 + 16
PW = 1568           # local (pre-shifted) positional width
BRS = 1280          # bd dram row stride (elements)
BDSZ = 5 * 128 * BRS
TAILOFF = 4 * 128 * (BRS - 1) + 96
NCORES = 8

PAIRS = [[0, 1], [2, 3], [4, 5], [6, 7]]
ALLG = [[0, 1, 2, 3, 4, 5, 6, 7]]

# bd q-chunks: (q0, qw, band_base, band_pi_width)
BDCH = [(0, 128, 416, 384), (128, 128, 288, 384), (256, 128, 160, 384),
        (384, 128, 32, 384), (512, 32, 0, 352)]


def _r(ap):
    return ap.bitcast(F32R)


def _emit(nc):
    def inp(name, shape, dt=F32):
        return nc.dram_tensor(name, list(shape), dt, kind="ExternalInput")

    x_d = inp("x_loc", (4, 128, HT), F32R)
    wf1_d = inp("wf1", (16, 128, D), BF16); bf1_d = inp("bf1", (128, 16))
    wf2_d = inp("wf2", (16, 128, D), BF16); bf2_d = inp("bf2", (128, 4))
    wq_d = inp("wq", (4, 128, D), BF16); bq_d = inp("bq", (128, 4))
    dqv_d = inp("dqv", (128, 4))
    wk_d = inp("wk", (4, 128, D), BF16); bk_d = inp("bk", (128, 4))
    wv_d = inp("wv", (4, 128, D), BF16); bv_d = inp("bvrow", (1, D))
    wp_d = inp("wp", (4, 128, D), BF16)
    wo_d = inp("wo", (4, 128, D), F32R); bo_d = inp("bo", (128, 4))
    posT_d = inp("posT", (4, 128, PW), BF16)
    pw1_d = inp("pw1", (4, 128, 1024), BF16); bpw1_d = inp("bpw1", (128, 8))
    dwdg_d = inp("dwdiag", (4, KCONV, 128, 128), BF16)
    bng_d = inp("bng", (128, 4)); bnb_d = inp("bnb", (128, 4))
    pw2_d = inp("pw2", (4, 128, D), BF16); bpw2_d = inp("bpw2", (128, 4))
    cmask_d = inp("cmask", (1, WQ))
    wg1_d = inp("wg1", (16, 128, D), BF16); bg1_d = inp("bg1", (128, 16))
    wg2_d = inp("wg2", (16, 128, D), BF16); bg2_d = inp("bg2", (128, 4))
    g5_d = inp("g5", (128, 4)); b5_d = inp("b5", (128, 4))
    onc_d = inp("onc", (128, 1), F32R)
    idb_d = inp("idb", (128, 128), BF16)
    onv_d = inp("onv", (128, 64), BF16)

    out_d = nc.dram_tensor("out_loc", [4, 128, HT], F32, kind="ExternalOutput")

    cc_h_in = nc.dram_tensor("cc_h_in", [4, 128, HT], F32R)
    cc_h_out = nc.dram_tensor("cc_h_out", [8, 128, HT], F32R)
    cc_bn_in = nc.dram_tensor("cc_bn_in", [128, 8], F32)
    cc_bn_out = nc.dram_tensor("cc_bn_out", [128, 8], F32)
    bd_d = [nc.dram_tensor(f"bd_{h}", [BDSZ], BF16) for h in range(H)]

    ln_uid = [0]

    with tile.TileContext(nc) as tc, ExitStack() as ctx:
        const = ctx.enter_context(tc.tile_pool(name="const", bufs=1))
        ones_c = const.tile([128, 1], F32R)
        nc.sync.dma_start(out=ones_c[:], in_=onc_d[:])
        eps1 = const.tile([1, 1], F32); nc.vector.memset(eps1[:], EPS)
        epsP = const.tile([128, 1], F32); nc.vector.memset(epsP[:], EPS)
        identb = const.tile([128, 128], BF16)
        nc.sync.dma_start(out=identb[:], in_=idb_d[:])

        late = ctx.enter_context(tc.tile_pool(name="late", bufs=1))
        h2w = late.tile([128, 4, WQ], F32R)   # post-attention hidden window
        h3_sb = late.tile([128, 4, HT], F32R)    # post-conv hidden

        # conv weights: tiles allocated up-front, DMAs issued mid-attention
        g12pool = tc.tile_pool(name="g12", bufs=1)
        g12 = g12pool.__enter__()
        w1c = g12.tile([128, 4, 1024], BF16)
        w2c = g12.tile([128, 4, D], BF16)
        dgt = g12.tile([128, 4, KCONV, 128], BF16)

        actpool = tc.tile_pool(name="act", bufs=1)
        act = actpool.__enter__()
        # full padded hidden frame (post-FFN1), 16 zero tokens on both ends
        hfe_sb = act.tile([128, 4, FR], F32R)
        nc.vector.memset(hfe_sb[:, :, 0:16].bitcast(F32), 0.0)
        nc.vector.memset(hfe_sb[:, :, 16 + T:].bitcast(F32), 0.0)

        pid = nc.vector.partition_id()
        woff = (pid % 2) * 512      # window start in frame coords

        # ---------- LN helper (channels-first) ----------
        def emit_ln(x4, out4, W, blocks, sbp, post=None):
            ln_uid[0] += 1
            with tc.tile_pool(name=f"lnps{ln_uid[0]}", bufs=1,
                              space="PSUM") as lnps:
                for b0, bw in blocks:
                    x2 = sbp.tile([128, 4, bw], F32R, tag="ln_sq")
                    for c in range(4):
                        nc.vector.tensor_mul(x2[:, c, :],
                                             x4[:, c, b0:b0 + bw],
                                             x4[:, c, b0:b0 + bw])
                    pss = lnps.tile([1, bw], F32, tag="lns")
                    psq = lnps.tile([1, bw], F32, tag="lnq")
                    for c in range(4):
                        nc.tensor.matmul(pss[:], _r(ones_c[:]),
                                         _r(x4[:, c, b0:b0 + bw]),
                                         start=(c == 0), stop=(c == 3))
                    for c in range(4):
                        nc.tensor.matmul(psq[:], _r(ones_c[:]),
                                         _r(x2[:, c, :]),
                                         start=(c == 0), stop=(c == 3))
                    mng = sbp.tile([1, bw], F32, tag="ln_m")
                    nc.scalar.activation(mng[:], pss[:], AF.Copy,
                                         scale=-1.0 / D)
                    var = sbp.tile([1, bw], F32, tag="ln_var")
                    nc.scalar.activation(var[:], psq[:], AF.Copy, scale=1.0 / D)
                    m2 = sbp.tile([1, bw], F32, tag="ln_sd")
                    nc.vector.tensor_mul(m2[:], mng[:], mng[:])
                    nc.vector.tensor_sub(var[:], var[:], m2[:])
                    nc.scalar.activation(m2[:], var[:], AF.Sqrt, bias=eps1[:])
                    rec = sbp.tile([1, bw], F32, tag="ln_rs")
                    nc.vector.reciprocal(rec[:], m2[:])
                    nc.vector.tensor_mul(mng[:], mng[:], rec[:])
                    rb_t = sbp.tile([128, bw], F32, tag="ln_rb")
                    nc.gpsimd.partition_broadcast(rb_t[:], rec[:])
                    nb_t = sbp.tile([128, bw], F32, tag="ln_nb")
                    nc.gpsimd.partition_broadcast(nb_t[:], mng[:])
                    for c in range(4):
                        o = out4[:, c, b0:b0 + bw]
                        nc.vector.tensor_mul(o, x4[:, c, b0:b0 + bw], rb_t[:])
                        nc.vector.tensor_add(o, o, nb_t[:])
                        if post is not None:
                            post(c, b0, bw)

        # ---------- FFN helper (weights already in SBUF, j-sliced) ----------
        def emit_ffn(xn, xres, out, w1, b1, w2, b2, pref):
            # software-pipelined: psy[j+1] is emitted before psh[j] so the
            # in-order PE stream never waits on the silu of the current j
            with tc.tile_pool(name=pref + "t", bufs=3) as tp, \
                 tc.tile_pool(name=pref + "ps", bufs=2, space="PSUM") as psp, \
                 tc.tile_pool(name=pref + "ph", bufs=1, space="PSUM") as php:
                psh = php.tile([128, 4, HT], F32)
                y1s = [None] * 16

                def psy_stage(j):
                    psy = psp.tile([128, HT], F32, tag="psy")
                    for c in range(4):
                        nc.tensor.matmul(psy[:], w1[:, j, ts(c, 128)],
                                         xn[:, c, :],
                                         start=(c == 0), stop=(c == 3))
                    y1 = tp.tile([128, HT], BF16, tag="y1")
                    nc.scalar.activation(y1[:], psy[:], AF.Silu,
                                         bias=b1[:, j:j + 1])
                    y1s[j] = y1

                psy_stage(0)
                for j in range(16):
                    if j + 1 < 16:
                        psy_stage(j + 1)
                    for f in range(4):
                        nc.tensor.matmul(psh[:, f, :],
                                         w2[:, j, ts(f, 128)], y1s[j][:],
                                         start=(j == 0), stop=(j == 15))
                for c in range(4):
                    nc.vector.scalar_tensor_tensor(
                        out=out[:, c, :], in0=psh[:, c, :],
                        scalar=b2[:, c:c + 1], in1=xres[:, c, :],
                        op0=ALU.add, op1=ALU.add)

        # ================= Stage A: FFN1 on local tokens =================
        with tc.tile_pool(name="f1w", bufs=1) as f1w, \
             tc.tile_pool(name="stA", bufs=1) as stA, \
             tc.tile_pool(name="stAt", bufs=1) as stAt:
            x_sb = stA.tile([128, 4, HT], F32R)
            nc.sync.dma_start(out=x_sb[:],
                              in_=x_d[:].rearrange("c p w -> p c w"))
            w1 = f1w.tile([128, 16, D], BF16)
            w2 = f1w.tile([128, 16, D], BF16)
            for g in range(4):
                nc.sync.dma_start(
                    out=w1[:, 4 * g:4 * g + 4, :],
                    in_=wf1_d[4 * g:4 * g + 4].rearrange("j p w -> p j w"))
                nc.scalar.dma_start(
                    out=w2[:, 4 * g:4 * g + 4, :],
                    in_=wf2_d[4 * g:4 * g + 4].rearrange("j p w -> p j w"))
            b1 = f1w.tile([128, 16], F32)
            nc.sync.dma_start(out=b1[:], in_=bf1_d[:])
            b2 = f1w.tile([128, 4], F32)
            nc.sync.dma_start(out=b2[:], in_=bf2_d[:])
            h_sb = stA.tile([128, 4, HT], F32R)
            xn1 = stA.tile([128, 4, HT], BF16)
            emit_ln(x_sb[:], xn1[:], HT, [(0, HT)], stAt)
            emit_ffn(xn1[:], x_sb[:], h_sb[:], w1, b1[:], w2, b2[:], "f1")
            nc.gpsimd.dma_start(
                out=cc_h_in[:].rearrange("c p w -> p c w"), in_=h_sb[:])

        nc.gpsimd.collective_compute(
            "AllGather", ALU.bypass, ins=[cc_h_in[:]], outs=[cc_h_out[:]],
            replica_groups=PAIRS)

        # ============ attention (all 8 heads, own 544-token window) =======
        attpool = tc.tile_pool(name="attp", bufs=1)
        attp = attpool.__enter__()
        p_sb = attp.tile([128, 4, PW], BF16)
        # ---- p-projection (independent of AllGather; overlaps it) ----
        with tc.tile_pool(name="ppp", bufs=2, space="PSUM") as ppp, \
             tc.tile_pool(name="atw", bufs=1) as atw:
            wp_sb = atw.tile([128, 4, D], BF16, tag="wp")
            nc.sync.dma_start(out=wp_sb[:],
                              in_=wp_d[:].rearrange("c p w -> p c w"))
            pos_sb = atw.tile([128, 4, PW], BF16, tag="pos")
            nc.scalar.dma_start(out=pos_sb[:],
                                in_=posT_d[:].rearrange("c p w -> p c w"))
            for hp in range(4):
                psp_t = ppp.tile([128, 3, 512], F32, tag="psp")
                psp_u = ppp.tile([128, 32], F32, tag="pspu")
                for c in range(4):
                    st, sp = (c == 0), (c == 3)
                    for pc in range(3):
                        nc.tensor.matmul(
                            psp_t[:, pc, :], wp_sb[:, c, ts(hp, 128)],
                            pos_sb[:, c, ts(pc, 512)], start=st, stop=sp)
                    nc.tensor.matmul(
                        psp_u[:], wp_sb[:, c, ts(hp, 128)],
                        pos_sb[:, c, 1536:PW], start=st, stop=sp)
                for pc in range(3):
                    if hp % 2 == 0:
                        nc.vector.tensor_copy(
                            p_sb[:, hp, ts(pc, 512)],
                            psp_t[:, pc, :])
                    else:
                        nc.scalar.activation(
                            p_sb[:, hp, ts(pc, 512)],
                            psp_t[:, pc, :], AF.Copy)
                nc.vector.tensor_copy(p_sb[:, hp, 1536:PW],
                                      psp_u[:])

        # ---- gather h_full into frame, LN2 over padded frame ----
        for half in range(2):
            nc.gpsimd.dma_start(
                out=hfe_sb[:, :, 16 + half * HT:16 + (half + 1) * HT],
                in_=cc_h_out[half * 4:half * 4 + 4].rearrange(
                    "c p w -> p c w"))
        v65pool = tc.tile_pool(name="v65p", bufs=1)
        v65pl = v65pool.__enter__()
        v65 = v65pl.tile([128, 8, 8, 65], BF16)
        xn2pool = tc.tile_pool(name="xn2", bufs=1)
        xn2pl = xn2pool.__enter__()
        xn2p = xn2pl.tile([128, 4, FR], BF16)
        with tc.tile_pool(name="stC", bufs=1) as stC, \
             tc.tile_pool(name="stCt", bufs=2) as stCt:
            emit_ln(hfe_sb[:], xn2p[:], FR,
                    [(0, 352), (352, 352), (704, 352)], stCt)

            # window copy for q (dynamic slice -> static tile)
            xn2w = stC.tile([128, 4, WQ], BF16)
            for c in range(4):
                nc.vector.tensor_copy(xn2w[:, c, :],
                                      xn2p[:, c, ds(woff, WQ)])

            # ---- q, qv, k, v projections ----
            q_sb = attp.tile([128, 4, WQ], BF16)
            qv_sb = attp.tile([128, 4, WQ], BF16)
            k_sb = attp.tile([128, 4, T], BF16)
            nc.sync.dma_start(out=v65[:, :, :, 64:65],
                              in_=onv_d[:].rearrange("p (a b) -> p a b", a=8))
            with tc.tile_pool(name="qkp", bufs=2, space="PSUM") as psp:
                wq_sb = stC.tile([128, 4, D], BF16)
                wk_sb = stC.tile([128, 4, D], BF16)
                wv_sb = xn2pl.tile([128, 4, D], BF16)
                nc.sync.dma_start(out=wq_sb[:],
                                  in_=wq_d[:].rearrange("c p w -> p c w"))
                nc.scalar.dma_start(out=wk_sb[:],
                                    in_=wk_d[:].rearrange("c p w -> p c w"))
                nc.sync.dma_start(out=wv_sb[:],
                                  in_=wv_d[:].rearrange("c p w -> p c w"))
                bq_sb = stC.tile([128, 4], F32)
                nc.sync.dma_start(out=bq_sb[:], in_=bq_d[:])
                bk_sb = stC.tile([128, 4], F32)
                nc.sync.dma_start(out=bk_sb[:], in_=bk_d[:])
                dqv_sb = stC.tile([128, 4], F32)
                nc.sync.dma_start(out=dqv_sb[:], in_=dqv_d[:])
                bv_sb = stC.tile([1, D], F32)
                nc.sync.dma_start(out=bv_sb[:], in_=bv_d[:])
                bvb_sb = stC.tile([128, D], F32)
                nc.gpsimd.partition_broadcast(bvb_sb[:], bv_sb[:])
                for hp in range(4):
                    psq = psp.tile([128, 2, 512], F32, tag="psq")
                    for c in range(4):
                        st, sp = (c == 0), (c == 3)
                        nc.tensor.matmul(psq[:, 0, :],
                                         wq_sb[:, c, ts(hp, 128)],
                                         xn2w[:, c, 0:512],
                                         start=st, stop=sp,
                                         skip_group_check=True)
                        nc.tensor.matmul(psq[:, 1, 0:32],
                                         wq_sb[:, c, ts(hp, 128)],
                                         xn2w[:, c, 512:WQ],
                                         start=st, stop=sp,
                                         skip_group_check=True)
                    nc.vector.tensor_scalar_add(
                        q_sb[:, hp, 0:512], psq[:, 0, :], bq_sb[:, hp:hp + 1])
                    nc.vector.tensor_scalar_add(
                        q_sb[:, hp, 512:WQ], psq[:, 1, 0:32],
                        bq_sb[:, hp:hp + 1])
                    nc.vector.tensor_scalar_add(qv_sb[:, hp, :],
                                                q_sb[:, hp, :],
                                                dqv_sb[:, hp:hp + 1])
                    for th in range(2):
                        psk = psp.tile([128, 512], F32, tag="psk")
                        for c in range(4):
                            nc.tensor.matmul(
                                psk[:], wk_sb[:, c, ts(hp, 128)],
                                xn2p[:, c, 16 + th * 512:16 + th * 512 + 512],
                                start=(c == 0), stop=(c == 3))
                        nc.vector.tensor_scalar_add(
                            k_sb[:, hp, th * 512:(th + 1) * 512], psk[:],
                            bk_sb[:, hp:hp + 1])
                for tq in range(8):
                    psv = psp.tile([128, 512], F32, tag="psv")
                    for c in range(4):
                        nc.tensor.matmul(
                            psv[:], xn2p[:, c, 16 + tq * 128:16 + (tq + 1) * 128],
                            wv_sb[:, c, :],
                            start=(c == 0), stop=(c == 3))
                    nc.vector.tensor_add(
                        v65[:, tq, :, 0:64],
                        psv[:].rearrange("p (h d) -> p h d", h=8),
                        bvb_sb[:].rearrange("p (h d) -> p h d", h=8))

        xn2pool.__exit__(None, None, None)

        # ---- per-head: banded bd -> DRAM bounce -> scores -> av ----
        o_h = attp.tile([128, 4, WQ], F32R)
        with tc.tile_pool(name="bds", bufs=2) as bdp, \
             tc.tile_pool(name="bdsh", bufs=2) as shp, \
             tc.tile_pool(name="atp", bufs=2) as atp, \
             tc.tile_pool(name="atn", bufs=1) as atn, \
             tc.tile_pool(name="psb", bufs=2, space="PSUM") as psb, \
             tc.tile_pool(name="pss", bufs=2, space="PSUM") as pss, \
             tc.tile_pool(name="psav", bufs=1, space="PSUM") as psav:

            def emit_bd(h):
                hc, hr = h // 2, 64 * (h % 2)
                for ci, (q0, qw, m0, pw) in enumerate(BDCH):
                    bdw = bdp.tile([128, 3, 384], BF16, tag="bdw")
                    for pi in range(3):
                        psB = psb.tile([128, 384], F32, tag="psB")
                        nc.tensor.matmul(
                            psB[0:qw, 0:pw],
                            qv_sb[hr:hr + 64, hc, q0:q0 + qw],
                            p_sb[hr:hr + 64, hc,
                                 m0 + pi * pw:m0 + (pi + 1) * pw],
                            start=True, stop=True)
                        if pi != 1:
                            nc.vector.tensor_copy(bdw[0:qw, pi, 0:pw],
                                                  psB[0:qw, 0:pw])
                        else:
                            nc.scalar.activation(bdw[0:qw, pi, 0:pw],
                                                 psB[0:qw, 0:pw], AF.Copy)
                    off = ci * 128 * (BRS - 1) if qw == 128 else TAILOFF
                    dst = bass.AP(tensor=bd_d[h], offset=off,
                                  ap=[[BRS, qw], [1, 3 * pw]])
                    nc.gpsimd.dma_start(
                        out=dst, in_=bdw[0:qw, :, 0:pw])

            def emit_scores(h, bdsh):
                hc, hr = h // 2, 64 * (h % 2)
                psA = psav.tile([65, 2, 512], F32, tag="psAV")
                probss = [None] * 8
                SL2 = [(0, slice(0, 512), slice(0, 512)),
                       (1, slice(0, 32), slice(512, WQ))]

                def sc_stage(kc):
                    psS = pss.tile([128, 2, 512], F32, tag="psS")
                    for h2, psl, qsl in SL2:
                        nc.tensor.matmul(
                            psS[:, h2, psl],
                            k_sb[hr:hr + 64, hc, ts(kc, 128)],
                            q_sb[hr:hr + 64, hc, qsl],
                            start=True, stop=False, skip_group_check=True)
                        nc.tensor.matmul(
                            psS[:, h2, psl], identb[:], bdsh[:, kc, qsl],
                            start=False, stop=True, skip_group_check=True)
                    probs = atp.tile([128, WQ], BF16, tag="probs")
                    nc.scalar.activation(probs[:, 0:512], psS[:, 0, :], AF.Exp)
                    nc.scalar.activation(probs[:, 512:WQ], psS[:, 1, 0:32],
                                         AF.Exp)
                    probss[kc] = probs

                sc_stage(0)
                for kc in range(8):
                    if kc + 1 < 8:
                        sc_stage(kc + 1)
                    for h2, psl, qsl in SL2:
                        nc.tensor.matmul(
                            psA[:, h2, psl], v65[:, kc, h, :],
                            probss[kc][:, qsl],
                            start=(kc == 0), stop=(kc == 7),
                            skip_group_check=True)
                s64 = atn.tile([65, 2, 512], F32, tag="s65")
                nc.scalar.activation(s64[64:65, :, :], psA[64:65, :, :],
                                     AF.Copy)
                row = atn.tile([1, 2, 512], F32, tag="row")
                nc.gpsimd.dma_start(out=row[:], in_=s64[64:65, :, :])
                rec = atn.tile([1, 2, 512], F32, tag="rec")
                scr = atn.tile([1, 2, 512], F32, tag="scr")
                nc.vector.reciprocal_approx_accurate(rec[:], row[:], scr[:])
                rb = atn.tile([64, 2, 512], F32, tag="rb")
                nc.gpsimd.partition_broadcast(rb[:], rec[:])
                nc.vector.tensor_mul(o_h[hr:hr + 64, hc, 0:512],
                                     psA[0:64, 0, :], rb[:, 0, :])
                nc.vector.tensor_mul(o_h[hr:hr + 64, hc, 512:WQ],
                                     psA[0:64, 1, 0:32], rb[:, 1, 0:32])

            def bdread(h):
                # XBAR-transposed read: [q, k]-contiguous rows -> [k, q] tile
                bdsh = shp.tile([128, 8, WQ], BF16, tag="sh")
                for kc in range(8):
                    src = bass.AP(tensor=bd_d[h], offset=127 + 128 * kc,
                                  ap=[[BRS - 1, WQ], [1, 128]])
                    nc.sync.dma_start_transpose(bdsh[:, kc, :], src)
                return bdsh

            emit_bd(0)
            bdsh_t = bdread(0)
            for h in range(H):
                if h + 1 < H:
                    emit_bd(h + 1)
                    nxt = bdread(h + 1)
                else:
                    nxt = None
                if h == 3:
                    nc.gpsimd.dma_start(
                        out=w1c[:], in_=pw1_d[:].rearrange("c p w -> p c w"))
                    nc.gpsimd.dma_start(
                        out=w2c[:], in_=pw2_d[:].rearrange("c p w -> p c w"))
                if h == 5:
                    nc.gpsimd.dma_start(
                        out=dgt[:].rearrange("p c j w -> p (c j) w"),
                        in_=dwdg_d[:].rearrange("c j p w -> p (c j) w"))
                emit_scores(h, bdsh_t)
                bdsh_t = nxt

        v65pool.__exit__(None, None, None)

        # ---- out-projection + residual (all local) ----
        with tc.tile_pool(name="pso", bufs=2, space="PSUM") as psop, \
             tc.tile_pool(name="aot", bufs=1) as aot:
            wo_sb = aot.tile([128, 4, D], F32R, tag="wo")
            nc.sync.dma_start(out=wo_sb[:],
                              in_=wo_d[:].rearrange("c p w -> p c w"))
            bo_sb = aot.tile([128, 4], F32)
            nc.sync.dma_start(out=bo_sb[:], in_=bo_d[:])
            for f in range(4):
                pso = psop.tile([128, 2, 512], F32, tag="pso")
                for hc in range(4):
                    st, sp = (hc == 0), (hc == 3)
                    nc.tensor.matmul(pso[:, 0, :],
                                     _r(wo_sb[:, hc, ts(f, 128)]),
                                     _r(o_h[:, hc, 0:512]),
                                     start=st, stop=sp,
                                     skip_group_check=True)
                    nc.tensor.matmul(pso[:, 1, 0:32],
                                     _r(wo_sb[:, hc, ts(f, 128)]),
                                     _r(o_h[:, hc, 512:WQ]),
                                     start=st, stop=sp,
                                     skip_group_check=True)
                nc.vector.scalar_tensor_tensor(
                    out=h2w[:, f, 0:512], in0=pso[:, 0, :],
                    scalar=bo_sb[:, f:f + 1],
                    in1=hfe_sb[:, f, ds(woff, 512)].bitcast(F32),
                    op0=ALU.add, op1=ALU.add)
                nc.vector.scalar_tensor_tensor(
                    out=h2w[:, f, 512:WQ], in0=pso[:, 1, 0:32],
                    scalar=bo_sb[:, f:f + 1],
                    in1=hfe_sb[:, f, ds(woff + 512, 32)].bitcast(F32),
                    op0=ALU.add, op1=ALU.add)
        attpool.__exit__(None, None, None)
        actpool.__exit__(None, None, None)

        # ================= conv module =================
        ggpool = tc.tile_pool(name="gg", bufs=1)
        gg = ggpool.__enter__()
        h4_sb = gg.tile([128, 4, HT], F32R)    # post-FFN2 hidden
        g1t = gg.tile([128, 16, D], BF16)
        g2t = gg.tile([128, 16, D], BF16)
        gb1 = gg.tile([128, 16], F32)
        gb2 = gg.tile([128, 4], F32)
        nc.scalar.dma_start(out=gb1[:], in_=bg1_d[:])
        nc.scalar.dma_start(out=gb2[:], in_=bg2_d[:])
        for g in range(4):
            nc.scalar.dma_start(
                out=g1t[:, 4 * g:4 * g + 4, :],
                in_=wg1_d[4 * g:4 * g + 4].rearrange("j p w -> p j w"))
        for g in range(4):
            nc.scalar.dma_start(
                out=g2t[:, 4 * g:4 * g + 4, :],
                in_=wg2_d[4 * g:4 * g + 4].rearrange("j p w -> p j w"))
        with tc.tile_pool(name="stF", bufs=1) as stF, \
             tc.tile_pool(name="stFt", bufs=1) as stFt:
            bp1 = stF.tile([128, 8], F32)
            nc.sync.dma_start(out=bp1[:], in_=bpw1_d[:])
            cm = stF.tile([1, WQ], F32)
            nc.sync.dma_start(out=cm[:], in_=cmask_d[:])
            cmb = stF.tile([128, WQ], F32)
            nc.gpsimd.partition_broadcast(cmb[:], cm[:])
            glu = stF.tile([128, 4, WQ], BF16)
            with tc.tile_pool(name="stF1", bufs=1) as stF1, \
                 tc.tile_pool(name="cvp1", bufs=1, space="PSUM") as cps:
                xn3 = stF1.tile([128, 4, WQ], BF16)
                emit_ln(h2w[:], xn3[:], WQ, [(0, 272), (272, 272)], stFt)
                for m in range(4):
                    psa = cps.tile([128, 2, 512], F32, tag="psa")
                    psg = cps.tile([128, 2, 512], F32, tag="psg")
                    for half in range(2):
                        sl = slice(half * 272, (half + 1) * 272)
                        for c in range(4):
                            nc.tensor.matmul(psa[:, half, 0:272],
                                             w1c[:, c, ts(m, 128)],
                                             xn3[:, c, sl],
                                             start=(c == 0), stop=(c == 3),
                                             skip_group_check=True)
                        for c in range(4):
                            nc.tensor.matmul(psg[:, half, 0:272],
                                             w1c[:, c, 512 + m * 128:
                                                 512 + (m + 1) * 128],
                                             xn3[:, c, sl],
                                             start=(c == 0), stop=(c == 3),
                                             skip_group_check=True)
                    sg = stFt.tile([128, WQ], F32, tag="sg")
                    sgv = sg[:].rearrange("p (a w) -> p a w", a=2)
                    gluv = glu[:, m, :].rearrange("p (a w) -> p a w", a=2)
                    nc.scalar.activation(sgv, psg[:, :, 0:272], AF.Sigmoid,
                                         bias=bp1[:, 4 + m:5 + m])
                    nc.vector.scalar_tensor_tensor(
                        out=gluv, in0=psa[:, :, 0:272], scalar=bp1[:, m:m + 1],
                        in1=sgv, op0=ALU.add, op1=ALU.mult)
                    nc.vector.tensor_mul(glu[:, m, :], glu[:, m, :], cmb[:])
            # depthwise conv: 31 accumulating diagonal bf16 matmuls per chunk
            acc = stF.tile([128, 4, HT], F32)
            bnpk = stF.tile([128, 8], F32)
            with tc.tile_pool(name="dgp", bufs=2, space="PSUM") as dgp:
                for c in range(4):
                    psC = dgp.tile([128, HT], F32, tag="psC")
                    for j in range(KCONV):
                        nc.tensor.matmul(psC[:], dgt[:, c, j, :],
                                         glu[:, c, 1 + j:1 + j + HT],
                                         start=(j == 0), stop=(j == KCONV - 1))
                    nc.scalar.activation(acc[:, c, :], psC[:], AF.Copy)
                    bst_t = stFt.tile([128, 6], F32, tag="bst")
                    nc.vector.bn_stats(bst_t[:], acc[:, c, :])
                    mv = stFt.tile([128, 2], F32, tag="mv")
                    nc.vector.bn_aggr(mv[:], bst_t[:])
                    nc.vector.tensor_copy(bnpk[:, 2 * c:2 * c + 1], mv[:, 0:1])
                    nc.vector.scalar_tensor_tensor(
                        out=bnpk[:, 2 * c + 1:2 * c + 2], in0=mv[:, 0:1],
                        scalar=mv[:, 0:1], in1=mv[:, 1:2],
                        op0=ALU.mult, op1=ALU.add)
            nc.sync.dma_start(out=cc_bn_in[:], in_=bnpk[:])
            nc.gpsimd.collective_compute(
                "AllReduce", ALU.add, ins=[cc_bn_in[:]],
                outs=[cc_bn_out[:]], replica_groups=ALLG)
            bnar8 = stF.tile([128, 8], F32)
            nc.sync.dma_start(out=bnar8[:], in_=cc_bn_out[:])
            bng_sb = stF.tile([128, 4], F32)
            nc.sync.dma_start(out=bng_sb[:], in_=bng_d[:])
            bnb_sb = stF.tile([128, 4], F32)
            nc.sync.dma_start(out=bnb_sb[:], in_=bnb_d[:])
            bp2 = stF.tile([128, 4], F32)
            nc.sync.dma_start(out=bp2[:], in_=bpw2_d[:])
            ysl = stF.tile([128, 4, HT], BF16)
            for c in range(4):
                mg = stFt.tile([128, 1], F32, tag="mg")
                nc.scalar.activation(mg[:], bnar8[:, 2 * c:2 * c + 1], AF.Copy,
                                     scale=1.0 / NCORES)
                e2 = stFt.tile([128, 1], F32, tag="e2c")
                nc.scalar.activation(e2[:], bnar8[:, 2 * c + 1:2 * c + 2],
                                     AF.Copy, scale=1.0 / NCORES)
                vg = stFt.tile([128, 1], F32, tag="vg")
                nc.vector.tensor_mul(vg[:], mg[:], mg[:])
                nc.vector.tensor_sub(vg[:], e2[:], vg[:])
                sdc = stFt.tile([128, 1], F32, tag="sdc")
                nc.scalar.activation(sdc[:], vg[:], AF.Sqrt, bias=epsP[:])
                rs = stFt.tile([128, 1], F32, tag="rsc")
                nc.vector.reciprocal(rs[:], sdc[:])
                s1 = stFt.tile([128, 1], F32, tag="s1c")
                nc.vector.tensor_mul(s1[:], rs[:], bng_sb[:, c:c + 1])
                s2 = stFt.tile([128, 1], F32, tag="s2c")
                nc.vector.tensor_mul(s2[:], mg[:], s1[:])
                nc.vector.tensor_sub(s2[:], bnb_sb[:, c:c + 1], s2[:])
                sg2 = stFt.tile([128, HT], F32, tag="sg2")
                nc.scalar.activation(sg2[:], acc[:, c, :], AF.Sigmoid,
                                     scale=s1[:], bias=s2[:])
                nc.vector.tensor_scalar(ysl[:, c, :], acc[:, c, :],
                                        s1[:], s2[:], ALU.mult, ALU.add)
                nc.vector.tensor_mul(ysl[:, c, :], ysl[:, c, :], sg2[:])
            with tc.tile_pool(name="cvp2", bufs=2, space="PSUM") as cps2:
                for f in range(4):
                    psw = cps2.tile([128, HT], F32, tag="psw")
                    for c in range(4):
                        nc.tensor.matmul(psw[:], w2c[:, c, ts(f, 128)],
                                         ysl[:, c, :],
                                         start=(c == 0), stop=(c == 3))
                    nc.vector.scalar_tensor_tensor(
                        out=h3_sb[:, f, :], in0=psw[:], scalar=bp2[:, f:f + 1],
                        in1=h2w[:, f, 16:16 + HT], op0=ALU.add, op1=ALU.add)

        # ================= FFN2 =================
        with tc.tile_pool(name="stG", bufs=1) as stG, \
             tc.tile_pool(name="stGt", bufs=1) as stGt:
            xn4 = stG.tile([128, 4, HT], BF16)
            emit_ln(h3_sb[:], xn4[:], HT, [(0, HT)], stGt)
            emit_ffn(xn4[:], h3_sb[:], h4_sb[:], g1t, gb1[:], g2t, gb2[:],
                     "f2")

        # ================= LN5 + output =================
        with tc.tile_pool(name="stH", bufs=1) as stH, \
             tc.tile_pool(name="stHt", bufs=2) as stHt:
            g5_sb = stH.tile([128, 4], F32)
            nc.sync.dma_start(out=g5_sb[:], in_=g5_d[:])
            b5_sb = stH.tile([128, 4], F32)
            nc.sync.dma_start(out=b5_sb[:], in_=b5_d[:])
            xn5 = stH.tile([128, 4, HT], F32)
            emit_ln(h4_sb[:], xn5[:], HT, [(0, HT)], stHt)
            for c in range(4):
                nc.vector.tensor_scalar(xn5[:, c, :], xn5[:, c, :],
                                        g5_sb[:, c:c + 1], b5_sb[:, c:c + 1],
                                        ALU.mult, ALU.add)
                nc.sync.dma_start(out=out_d[c], in_=xn5[:, c, :])
        ggpool.__exit__(None, None, None)
        g12pool.__exit__(None, None, None)
    return nc


_CACHE = {}


def build_nc():
    if "nc" not in _CACHE:
        nc = bacc.Bacc("TRN2", target_bir_lowering=False, debug=False,
                       num_devices=NCORES)
        _emit(nc)
        nc.compile()
        _CACHE["nc"] = nc
    return _CACHE["nc"]


def _chunk_cf(a2d):
    """[Dany, W] -> [Dany//128, 128, W] chunk-major channels-first."""
    d, w = a2d.shape
    return np.ascontiguousarray(a2d.reshape(d // 128, 128, w), dtype=np.float32)


def round_f32r(a):
    import ml_dtypes
    a = np.asarray(a, dtype=np.float32)
    hi = a.astype(ml_dtypes.bfloat16).astype(np.float32)
    lo = (a - hi).astype(ml_dtypes.bfloat16).astype(np.float32)
    return (hi + lo).astype(np.float32)


def _pcol(vec):
    """[Dout] per-channel vector -> [128, Dout//128] (partition, chunk)."""
    n = vec.shape[0]
    return np.ascontiguousarray(vec.reshape(n // 128, 128).T, dtype=np.float32)


def _jslice(w):
    """[D, DFF] -> [16, 128, 512] j-major stationary slices."""
    return np.ascontiguousarray(
        w.reshape(4, 128, 16, 128).transpose(2, 1, 0, 3).reshape(16, 128, 512),
        dtype=np.float32)


def make_in_maps(inputs):
    import ml_dtypes
    inputs = {k: np.asarray(v, dtype=np.float32) for k, v in inputs.items()}
    x = inputs["x"]; pos_emb = inputs["pos_emb"]
    ln1_g, ln1_b = inputs["ln1_g"], inputs["ln1_b"]
    ln2_g, ln2_b = inputs["ln2_g"], inputs["ln2_b"]
    ln3_g, ln3_b = inputs["ln3_g"], inputs["ln3_b"]
    ln4_g, ln4_b = inputs["ln4_g"], inputs["ln4_b"]
    ln5_g, ln5_b = inputs["ln5_g"], inputs["ln5_b"]

    w1f = ln1_g[:, None] * inputs["ff1_w1"]
    b1f = inputs["ff1_b1"] + ln1_b @ inputs["ff1_w1"]
    w2f = 0.5 * inputs["ff1_w2"]; b2f = 0.5 * inputs["ff1_b2"]
    wg1f = ln4_g[:, None] * inputs["ff2_w1"]
    bg1f = inputs["ff2_b1"] + ln4_b @ inputs["ff2_w1"]
    wg2f = 0.5 * inputs["ff2_w2"]; bg2f = 0.5 * inputs["ff2_b2"]

    s = DK ** -0.5
    pos_u_f = inputs["pos_u"].reshape(D); pos_v_f = inputs["pos_v"].reshape(D)
    wqf = s * (ln2_g[:, None] * inputs["wq"])
    bqf = s * (inputs["bq"] + ln2_b @ inputs["wq"] + pos_u_f)
    dqvf = s * (pos_v_f - pos_u_f)
    wkf = ln2_g[:, None] * inputs["wk"]
    bkf = inputs["bk"] + ln2_b @ inputs["wk"]
    wvf = ln2_g[:, None] * inputs["wv"]
    bvf = inputs["bv"] + ln2_b @ inputs["wv"]

    # positional frame, padded: 16 left + 2047 + 49 right
    p_pad = np.zeros((D, 2112), dtype=np.float32)
    p_pad[:, 16:16 + 2 * T - 1] = pos_emb[0].T

    pw1f = (inputs["pw1_w"] * ln3_g[None, :]).T            # [512, 1024]
    bpw1f = inputs["pw1_b"] + inputs["pw1_w"] @ ln3_b      # [1024]
    dwwf = inputs["dw_w"][:, 0, :]                         # [512, 31]
    dwdiag = np.zeros((4, KCONV, 128, 128), dtype=np.float32)
    ar = np.arange(128)
    for c4 in range(4):
        for j4 in range(KCONV):
            dwdiag[c4, j4, ar, ar] = dwwf[c4 * 128:(c4 + 1) * 128, j4]
    pw2f = inputs["pw2_w"].T                               # [512, 512]

    bf = lambda a: a.astype(ml_dtypes.bfloat16)
    base = {
        "wf1": bf(_jslice(w1f)), "bf1": _pcol(b1f),
        "wf2": bf(_chunk_cf(w2f)), "bf2": _pcol(b2f),
        "wg1": bf(_jslice(wg1f)), "bg1": _pcol(bg1f),
        "wg2": bf(_chunk_cf(wg2f)), "bg2": _pcol(bg2f),
        "wq": bf(_chunk_cf(wqf)), "bq": _pcol(bqf),
        "dqv": _pcol(dqvf),
        "wk": bf(_chunk_cf(wkf)), "bk": _pcol(bkf),
        "wv": bf(_chunk_cf(wvf)),
        "bvrow": np.ascontiguousarray(bvf.reshape(1, D), dtype=np.float32),
        "wp": bf(_chunk_cf(inputs["wp"])),
        "wo": round_f32r(np.ascontiguousarray(
            inputs["wo"].reshape(4, 128, D), dtype=np.float32)),
        "bo": _pcol(inputs["bo"]),
        "pw1": bf(_chunk_cf(pw1f)), "bpw1": _pcol(bpw1f),
        "dwdiag": bf(dwdiag),
        "bng": _pcol(inputs["bn_g"]), "bnb": _pcol(inputs["bn_b"]),
        "pw2": bf(_chunk_cf(pw2f)), "bpw2": _pcol(inputs["pw2_b"]),
        "g5": _pcol(ln5_g), "b5": _pcol(ln5_b),
        "onc": np.ones((128, 1), dtype=np.float32),
        "idb": np.eye(128, dtype=np.float32).astype(ml_dtypes.bfloat16),
        "onv": np.ones((128, 64), dtype=ml_dtypes.bfloat16),
    }

    in_maps = []
    for c in range(NCORES):
        b, sc = c // 2, c % 2
        m = dict(base)
        xb = x[b, sc * HT:(sc + 1) * HT, :].T               # [512, 512]
        m["x_loc"] = round_f32r(_chunk_cf(xb))
        # per-core pre-shifted positional slice (band bases become static)
        lo = 512 - 512 * sc
        m["posT"] = bf(_chunk_cf(p_pad[:, lo:lo + PW]))
        cmask = np.ones((1, WQ), dtype=np.float32)
        if sc == 0:
            cmask[0, :16] = 0.0
        else:
            cmask[0, WQ - 16:] = 0.0
        m["cmask"] = cmask
        in_maps.append(m)
    return in_maps


def assemble_out(results):
    out = np.empty((B, T, D), dtype=np.float32)
    for c in range(NCORES):
        b, sc = c // 2, c % 2
        ol = np.asarray(results[c]["out_loc"])              # [4, 128, 512]
        out[b, sc * HT:(sc + 1) * HT, :] = ol.reshape(D, HT).T
    return out


def kernel(**inputs):
    in_maps = make_in_maps(inputs)
    nc = build_nc()
    res = run_bass_kernel_spmd(nc, in_maps, list(range(NCORES)))
    return assemble_out(res.results)
